# revision 17
# baseline (speedup 1.0000x reference)
"""Causal attention (B=4, S=2048, D=1024) on 8 trn2 NeuronCores.

Sharding: core c = (batch b = c//2, query-group h = c%2). Default scheme
(version 3): each core K-projects its OWN key half (pairwise AllGather of KT
hides behind the V projection), V-projects its whole batch locally, and
Q-projects its own 8 query tiles of 128 rows. Tiles are interleaved (t % 4
in {0,3} for h=0, {1,2} for h=1) so both cores of a pair have the same
causal work profile and the SPMD program is identical on every core.

All matmul operands are bf16 (fp32 PSUM accumulation): halves DMA bytes and
SBUF footprint vs f32r, so x / K / V / Q all stay SBUF-resident. Inputs are
host-prepacked into the exact SBUF tile layout [128, G, 8, 512] so every
load is one DMA with 128 contiguous per-partition runs (DIRECT2D descriptor
generation is serialized per sequencer and costs ~5ns/descriptor — layout,
batching, and spreading issuance across the sync/scalar/gpsimd queues keep
it off the critical path). Collective-dependent readbacks ride the gpsimd
queue: a sync-queue wait on an unfinished collective deadlocks.

Device kernel per core:
  KT[o,k] = sum_d WkT[d,o] xTk[d,k]         k = own 1024 keys, then
                                            pairwise AllGather -> all 2048
  V[s,o]  = sum_d xT[d,s]  WvT[d,o]         s = 0..2047 (local, duplicated)
  QT[o,q] = sum_d WqT[d,o] xTq[d,q]         q = core's 1024 rows
                                            (Wq pre-scaled by 1/32 on host)
  per sorted q-tile position j (L = (2j+2)*128 keys, both h fit under L):
    S[q,k] = sum_o QT[o,q] KT[o,k];  last 256 cols += mask (covers diag
             block + the 128-col overhang the other h-core doesn't need)
    P = exp(S)  (no rowmax subtraction: |S| <= ~6, exp is fp32-safe;
             masked cols are -1e30 -> exp underflows to exactly 0)
    rowsum fused via activation accum_out
    C[q,:] = sum_k P^T[k,q] V[k,:]  (P^T via PE transpose, bf16)
    out = C * (1/rowsum)
"""

import os
import sys
from contextlib import ExitStack

import ml_dtypes
import numpy as np

sys.path.insert(0, "/opt/trn_rl_repo")

import concourse.bass as bass
import concourse.tile as tile
from concourse import bacc, mybir
from concourse.bass_utils import run_bass_kernel_spmd

F32 = mybir.dt.float32
BF16 = mybir.dt.bfloat16
NPBF16 = ml_dtypes.bfloat16
P = 128
B, S, D = 4, 2048, 1024
NDC = D // P                     # 8 contraction chunks of 128
NQT = 8                          # q-tiles of 128 rows per core
QCORE = NQT * P                  # 1024 q rows per core
TILES = {
    0: [t for t in range(16) if t % 4 in (0, 3)],
    1: [t for t in range(16) if t % 4 in (1, 2)],
}
# position j covers L_j = (2j+2)*128 key columns: the max over the two
# h-cores' causal needs at that sorted position; the mask input zeroes the
# per-core overhang (at most 128 cols, always inside the last 256).
LJS = [(2 * j + 2) * P for j in range(NQT)]

_COMPILED = {}
LAST_RESULTS = None


def _score_chunks(L):
    """Split L key cols into matmul chunks <=512; last chunk is the 256-wide
    mask window."""
    pre = L - 256
    chunks = []
    off = 0
    while pre - off >= 512:
        chunks.append((off, 512, False))
        off += 512
    if pre - off:
        chunks.append((off, pre - off, False))
    chunks.append((pre, 256, True))
    return chunks


def _emit_body(nc, tc, rctx, aps, version=1):
    if version == 1:
        xT, xTq, wqT, wkT, wvT, masks, ident, out, pspool = aps
        cc = None
    elif version == 2:
        (xT, xTq, wqT, wkT, wvT, masks, ident, out,
         ktag_in, ktag_out, vag_in, vag_out, pspool) = aps
        cc = [[0, 1], [2, 3], [4, 5], [6, 7]]
    else:  # version 3: K gathered pairwise, V+Q local
        (xT, xTk, xTq, wqT, wkT, wvT, masks, ident, out,
         ktag_in, ktag_out, pspool) = aps
        cc = [[0, 1], [2, 3], [4, 5], [6, 7]]
    KH = S if version in (1, 3) else S // 2  # value rows projected locally
    KK = S // 2 if version in (2, 3) else S  # key rows projected locally
    copy_ctr = [0]

    def copy_out(dst, src):
        # alternate PSUM->SBUF copies between vector and scalar engines
        copy_ctr[0] += 1
        if copy_ctr[0] % 2:
            nc.vector.tensor_copy(dst, src)
        else:
            nc.scalar.copy(dst, src)

    cpool = rctx.enter_context(tc.tile_pool(name="const", bufs=1))
    identsb = cpool.tile([P, P], BF16)
    masksb = cpool.tile([P, NQT, 256], F32)
    ktpool = rctx.enter_context(tc.tile_pool(name="ktp", bufs=1))
    kt_sb = ktpool.tile([P, NDC, S], BF16)     # KT: [o%128, o//128, k]
    vpool = rctx.enter_context(tc.tile_pool(name="vp", bufs=1))
    v_sb = vpool.tile([P, S // P, D], BF16)    # V: [s%128, s//128, o]
    qtpool = rctx.enter_context(tc.tile_pool(name="qtp", bufs=1))
    qt_sb = qtpool.tile([P, NDC, QCORE], BF16)  # QT: [o%128, o//128, q]

    with tc.tile_pool(name="wts", bufs=1) as wpool:
        wv_sb = wpool.tile([P, 2, NDC, 512], BF16)
        wq_sb = wpool.tile([P, 2, NDC, 512], BF16)
        xt_sb = wpool.tile([P, KH // 512, NDC, 512], BF16)  # [p, s//512, d, s%512]

        # inputs are host-prepacked as [128, G, 8, 512] (exact SBUF tile
        # layout, contiguous per partition): each 512-col group is one DMA
        # with 128 contiguous runs -> cheap descriptor generation
        def ld(eng, dst, w, g):
            eng.dma_start(dst[:, g], w[:, g])

        # ---- K phase: wk/xtk/kstg live only here (SBUF headroom) ---------
        with tc.tile_pool(name="kph", bufs=1) as kpool:
            wk_sb = kpool.tile([P, 2, NDC, 512], BF16)
            if version == 3:
                xtk_sb = kpool.tile([P, KK // 512, NDC, 512], BF16)  # own-half xT
            else:
                xtk_sb = xt_sb

            # issue order = first-use order; spread across the sync /
            # scalar / vector HWDGE queues so descriptor generation
            # doesn't serialize behind one sequencer
            xk_src = xTk if version == 3 else xT
            ld(nc.sync, wk_sb, wkT, 0)
            ld(nc.sync, xtk_sb, xk_src, 0)
            ld(nc.sync, wk_sb, wkT, 1)
            for g in range(1, KK // 512):
                ld(nc.sync, xtk_sb, xk_src, g)
            if version == 3:
                for g in range(KH // 512):
                    ld(nc.scalar, xt_sb, xT, g)
            ld(nc.sync, wv_sb, wvT, 0)
            ld(nc.sync, wv_sb, wvT, 1)
            ld(nc.scalar, wq_sb, wqT, 0)
            ld(nc.scalar, wq_sb, wqT, 1)
            nc.scalar.dma_start(identsb[:], ident[:])
            nc.scalar.dma_start(masksb[:], masks[:])

            # ---- K projection: KT[o, own keys] ---------------------------
            for ks in range(KK // 512):
                if version in (2, 3):
                    kst = kpool.tile([P, NDC, 512], BF16, tag="kstg", bufs=2)
                for c in range(NDC):
                    ps = pspool.tile([P, 512], F32, tag="mm", bufs=3)
                    for d in range(NDC):
                        nc.tensor.matmul(
                            ps[:],
                            wk_sb[:, c // 4, d, (c % 4) * P : (c % 4 + 1) * P],
                            xtk_sb[:, ks, d, :],
                            start=(d == 0),
                            stop=(d == NDC - 1),
                        )
                    if version == 1:
                        copy_out(kt_sb[:, c, ks * 512 : (ks + 1) * 512], ps[:])
                    else:
                        copy_out(kst[:, c, :], ps[:])
                if version in (2, 3):
                    # stage this k-chunk to DRAM for the pairwise gather
                    nc.gpsimd.dma_start(
                        ktag_in[:, ks * 512 : (ks + 1) * 512].rearrange(
                            "(c p) k -> p c k", p=P
                        ),
                        kst[:],
                    )
            if version in (2, 3):
                nc.gpsimd.collective_compute(
                    "AllGather", mybir.AluOpType.bypass, replica_groups=cc,
                    ins=[ktag_in[:]], outs=[ktag_out[:]],
                )
            if version == 3:
                # kt readback on the idle gpsimd queue, right behind the
                # collective: no sync-queue sem wait, so V/Q-proj DMA waits
                # can't get serialized behind the gather
                for r in range(2):
                    nc.gpsimd.dma_start(
                        kt_sb[:, :, r * KK : (r + 1) * KK],
                        ktag_out[r * D : (r + 1) * D, :].rearrange(
                            "(c p) k -> p c k", p=P
                        ),
                    )

        # ---- post-K pool: xtq (+ v2 staging) in the freed space ----------
        with tc.tile_pool(name="qph", bufs=1) as qpool:
            xtq_sb = qpool.tile([P, 2, NDC, 512], BF16)
            if version == 2:
                vstg = qpool.tile([P, KH // P, D], BF16)  # staged V
            ld(nc.scalar, xtq_sb, xTq, 0)
            ld(nc.scalar, xtq_sb, xTq, 1)

            # ---- V projection: V[own rows, o] --------------------------------
            vdst = vstg if version == 2 else v_sb
            for st_i in range(KH // P):
                for oh in range(2):
                    ps = pspool.tile([P, 512], F32, tag="mm", bufs=3)
                    for d in range(NDC):
                        nc.tensor.matmul(
                            ps[:],
                            xt_sb[:, st_i // 4, d, (st_i % 4) * P : (st_i % 4 + 1) * P],
                            wv_sb[:, oh, d, :],
                            start=(d == 0),
                            stop=(d == NDC - 1),
                        )
                    copy_out(vdst[:, st_i, oh * 512 : (oh + 1) * 512], ps[:])
                if version == 2 and st_i == 3:
                    # first V half staged -> gather it while the second half
                    # computes; kt readback rides the gpsimd queue in between
                    # (K gather already done, so it doesn't block the V gather).
                    # Gather A rows = [s 0:512 | s 1024:1536] (rank-major).
                    nc.sync.dma_start(
                        vag_in[0:512, :].rearrange("(t p) o -> p t o", p=P),
                        vstg[:, 0:4, :],
                    )
                    nc.gpsimd.collective_compute(
                        "AllGather", mybir.AluOpType.bypass, replica_groups=cc,
                        ins=[vag_in[0:512, :]], outs=[vag_out[0:1024, :]],
                    )
                    # kt readback on gpsimd behind gather A (K gather long
                    # done); keeping collective-dependent waits off the sync
                    # queue — a sync wait on an unfinished collective deadlocks
                    for r in range(2):
                        nc.gpsimd.dma_start(
                            kt_sb[:, :, r * KH : (r + 1) * KH],
                            ktag_out[r * D : (r + 1) * D, :].rearrange(
                                "(c p) k -> p c k", p=P
                            ),
                        )
                    # vagA available: v s-tiles 0-3 and 8-11
                    nc.gpsimd.dma_start(
                        v_sb[:, 0:4, :],
                        vag_out[0:512, :].rearrange("(t p) o -> p t o", p=P),
                    )
                    nc.gpsimd.dma_start(
                        v_sb[:, 8:12, :],
                        vag_out[512:1024, :].rearrange("(t p) o -> p t o", p=P),
                    )
            if version == 2:
                # Gather B rows = [s 512:1024 | s 1536:2048]
                nc.sync.dma_start(
                    vag_in[512:1024, :].rearrange("(t p) o -> p t o", p=P),
                    vstg[:, 4:8, :],
                )
                nc.gpsimd.collective_compute(
                    "AllGather", mybir.AluOpType.bypass, replica_groups=cc,
                    ins=[vag_in[512:1024, :]], outs=[vag_out[1024:2048, :]],
                )

            # ---- Q projection: QT[o, q] --------------------------------------
            for qs in range(QCORE // 512):
                for c in range(NDC):
                    ps = pspool.tile([P, 512], F32, tag="mm", bufs=3)
                    for d in range(NDC):
                        nc.tensor.matmul(
                            ps[:],
                            wq_sb[:, c // 4, d, (c % 4) * P : (c % 4 + 1) * P],
                            xtq_sb[:, qs, d, :],
                            start=(d == 0),
                            stop=(d == NDC - 1),
                        )
                    copy_out(qt_sb[:, c, qs * 512 : (qs + 1) * 512], ps[:])

            if version == 2:
                # vagB readback: v s-tiles 4-7 first (needed by PV from j=2 on)
                nc.gpsimd.dma_start(
                    v_sb[:, 4:8, :],
                    vag_out[1024:1536, :].rearrange("(t p) o -> p t o", p=P),
                )
                nc.gpsimd.dma_start(
                    v_sb[:, 12:16, :],
                    vag_out[1536:2048, :].rearrange("(t p) o -> p t o", p=P),
                )

    # ---- attention, software-pipelined per q-tile position ---------------
    with tc.tile_pool(name="sp", bufs=2) as sp, tc.tile_pool(
        name="pp", bufs=2
    ) as pp, tc.tile_pool(name="stats", bufs=4) as stp, tc.tile_pool(
        name="atp", bufs=4
    ) as atp, tc.tile_pool(name="cp", bufs=2) as cp:
        state = {}

        def emit_scores(j):
            L = LJS[j]
            ssb = sp.tile([P, L], F32, tag="ssb", name=f"ssb{j}")
            for off, w, is_mask in _score_chunks(L):
                ps = pspool.tile([P, 512], F32, tag="mm", bufs=3)
                for c in range(NDC):
                    nc.tensor.matmul(
                        ps[:, :w],
                        qt_sb[:, c, j * P : (j + 1) * P],
                        kt_sb[:, c, off : off + w],
                        start=(c == 0),
                        stop=(c == NDC - 1),
                    )
                if is_mask:
                    nc.vector.tensor_add(
                        ssb[:, off : off + w], ps[:, :w], masksb[:, j, :]
                    )
                else:
                    copy_out(ssb[:, off : off + w], ps[:, :w])
            state[j] = ssb

        def emit_softmax_pv(j):
            L = LJS[j]
            ssb = state.pop(j)
            psb = pp.tile([P, L], BF16, tag="psb", name=f"psb{j}")
            sumv = stp.tile([P, 1], F32, tag="sumv", name=f"sumv{j}")
            nc.scalar.activation(
                psb[:],
                ssb[:],
                mybir.ActivationFunctionType.Exp,
                scale=1.0,
                accum_out=sumv[:],
            )
            rcp = stp.tile([P, 1], F32, tag="rcp", name=f"rcp{j}")
            nc.vector.reciprocal(rcp[:], sumv[:])

            co0 = pspool.tile([P, 512], F32, tag="co", bufs=2, name=f"co0_{j}")
            co1 = pspool.tile([P, 512], F32, tag="co", bufs=2, name=f"co1_{j}")
            nkt = L // P
            for k in range(nkt):
                tp = pspool.tile([P, P], BF16, tag="tp", bufs=3, name=f"tp{j}_{k}")
                nc.tensor.transpose(tp[:], psb[:, k * P : (k + 1) * P], identsb[:])
                at = atp.tile([P, P], BF16, tag="at", name=f"at{j}_{k}")
                copy_out(at[:], tp[:])
                nc.tensor.matmul(
                    co0[:], at[:], v_sb[:, k, 0:512],
                    start=(k == 0), stop=(k == nkt - 1),
                )
                nc.tensor.matmul(
                    co1[:], at[:], v_sb[:, k, 512:1024],
                    start=(k == 0), stop=(k == nkt - 1),
                )
            csb = cp.tile([P, D], F32, tag="csb", name=f"csb{j}")
            nc.vector.tensor_scalar_mul(csb[:, 0:512], co0[:], rcp[:])
            nc.sync.dma_start(out[j * P : (j + 1) * P, 0:512], csb[:, 0:512])
            nc.scalar.activation(
                csb[:, 512:1024],
                co1[:],
                mybir.ActivationFunctionType.Copy,
                scale=rcp[:],
            )
            nc.sync.dma_start(out[j * P : (j + 1) * P, 512:1024], csb[:, 512:1024])

        emit_scores(0)
        for j in range(1, NQT):
            emit_scores(j)
            emit_softmax_pv(j - 1)
        emit_softmax_pv(NQT - 1)


def _emit_body_v45(nc, tc, rctx, aps, version=4):
    """S^T-direct attention (no PE transposes). version 4: V projected
    locally for all 2048 rows (duplicated per pair, like v3). version 5:
    V projection sharded by rows + pairwise AllGather (like v2).

    Attention works on two quads of 4 sorted q-tile positions each. For
    quad g (positions p = 4g..4g+3), scores are computed TRANSPOSED:
      S^T[k, q] = sum_o KT[o, k-block] QT[o, q]   (k on partitions)
    per 128-row k-block kb, over the q columns of all members that
    causally need kb (member p needs kb <= 2p+1 across both h cores; the
    h-specific overhang + diagonal triangle are handled by a per-core
    maskT input added on DVE). exp(S^T) lands directly in the PV
    stationary layout, so the 72 PE transposes + PSUM->SBUF P^T copies
    of v3 disappear. The softmax row-sum rides a 1-column ones-matmul
    that reuses the already-loaded P^T stationary.
    """
    if version == 4:
        (xT, xTk, xTq, wqT, wkT, wvT, masksT, ones, out,
         ktag_in, ktag_out, pspool) = aps
    else:
        (xT, xTq, wqT, wkT, wvT, masksT, ones, out,
         ktag_in, ktag_out, vag_in, vag_out, pspool) = aps
    cc = [[0, 1], [2, 3], [4, 5], [6, 7]]
    KH = S if version == 4 else S // 2   # V rows projected locally
    KK = S // 2                          # K rows projected locally
    copy_ctr = [0]

    def copy_out(dst, src):
        copy_ctr[0] += 1
        if copy_ctr[0] % 2:
            nc.vector.tensor_copy(dst, src)
        else:
            nc.scalar.copy(dst, src)

    ktpool = rctx.enter_context(tc.tile_pool(name="ktp", bufs=1))
    # KT chunk-major: [o%128, key-chunk(512), o//128, k%512] so each
    # gather-readback DMA writes one contiguous chunk (no interleaved
    # address ranges -> no false deps stalling the first scores)
    kt_sb = ktpool.tile([P, 4, NDC, 512], BF16)
    vpool = rctx.enter_context(tc.tile_pool(name="vp", bufs=1))
    v_sb = vpool.tile([P, S // P, D], BF16)    # V: [s%128, s//128, o]
    qtpool = rctx.enter_context(tc.tile_pool(name="qtp", bufs=1))
    qt_sb = qtpool.tile([P, NDC, QCORE], BF16)  # QT: [o%128, o//128, q]

    with tc.tile_pool(name="wts", bufs=1) as wpool:
        wv_sb = wpool.tile([P, 2, NDC, 512], BF16)
        wq_sb = wpool.tile([P, 2, NDC, 512], BF16)
        xtq_sb = wpool.tile([P, 2, NDC, 512], BF16)
        xt_sb = wpool.tile([P, KH // 512, NDC, 512], BF16)

        def ld(eng, dst, w, g):
            eng.dma_start(dst[:, g], w[:, g])

        def ld2(eng_a, eng_b, dst, w, g):
            # split one 512-col group across two DMA queues (halves the
            # startup-critical load latency); only sync/scalar/gpsimd
            # have DMA queues
            eng_a.dma_start(dst[:, g, 0:4], w[:, g, 0:4])
            eng_b.dma_start(dst[:, g, 4:8], w[:, g, 4:8])

        # ---- K phase ----------------------------------------------------
        with tc.tile_pool(name="kph", bufs=1) as kpool:
            wk_sb = kpool.tile([P, 2, NDC, 512], BF16)
            if version == 4:
                xtk_sb = kpool.tile([P, KK // 512, NDC, 512], BF16)
                xk_src = xTk
            else:
                xtk_sb = xt_sb
                xk_src = xT

            # The 3 DMA queues each sustain only ~90GB/s and sit idle
            # for the first ~11us, and the kernel needs 14MB of input
            # streamed just-in-time. Emit ALL loads as 256KB 2-d-chunk
            # pieces, round-robin across queues in exact first-use
            # order. gpsimd takes only pieces needed before ~t=24us —
            # after that it is reserved for K staging + collective +
            # readbacks.
            def ldp(eng, dst, w, g, d0, d1):
                eng.dma_start(dst[:, g, d0:d1], w[:, g, d0:d1])

            early = [nc.sync, nc.scalar, nc.gpsimd]
            late = [nc.sync, nc.scalar]
            # (dst, src, group) in first-use order
            k_loads = [(wk_sb, wkT, 0), (xtk_sb, xk_src, 0),
                       (wk_sb, wkT, 1)]
            if version == 4:
                k_loads.append((xtk_sb, xTk, 1))
                v_loads = [(wv_sb, wvT, 0), (xt_sb, xT, 0),
                           (wv_sb, wvT, 1), (xt_sb, xT, 1),
                           (xt_sb, xT, 2), (xt_sb, xT, 3)]
            else:
                k_loads.append((xt_sb, xT, 1))
                v_loads = [(wv_sb, wvT, 0), (wv_sb, wvT, 1)]
            q_loads = [(wq_sb, wqT, 0), (xtq_sb, xTq, 0),
                       (wq_sb, wqT, 1), (xtq_sb, xTq, 1)]
            i = 0
            for dst, w, g in k_loads:
                for dd in range(4):
                    ldp(early[i % 3], dst, w, g, 2 * dd, 2 * dd + 2)
                    i += 1
            i = 0
            for dst, w, g in v_loads + q_loads:
                for dd in range(4):
                    ldp(late[i % 2], dst, w, g, 2 * dd, 2 * dd + 2)
                    i += 1

            # K projection: KT[o, own keys]; stage each 128-o chunk to
            # DRAM as soon as it is copied out so the gather can trigger
            # right after the last chunk
            for ks in range(KK // 512):
                kst = kpool.tile(
                    [P, NDC, 512], BF16, tag="kstg",
                    bufs=2 if version == 5 else 1,
                )
                for c in range(NDC):
                    ps = pspool.tile([P, 512], F32, tag="mm", bufs=3)
                    for d in range(NDC):
                        nc.tensor.matmul(
                            ps[:],
                            wk_sb[:, c // 4, d, (c % 4) * P : (c % 4 + 1) * P],
                            xtk_sb[:, ks, d, :],
                            start=(d == 0),
                            stop=(d == NDC - 1),
                        )
                    copy_out(kst[:, c, :], ps[:])
                    nc.gpsimd.dma_start(
                        ktag_in[c * P : (c + 1) * P, ks * 512 : (ks + 1) * 512],
                        kst[:, c, :],
                    )
            nc.gpsimd.collective_compute(
                "AllGather", mybir.AluOpType.bypass, replica_groups=cc,
                ins=[ktag_in[:]], outs=[ktag_out[:]],
            )
            # readback in 4 chunked DMAs; chunk ci = global keys
            # ci*512:(ci+1)*512 = ktag_out[rank(ci)*D rows, (ci%2) cols]
            for ci in range(4):
                r, half = ci // 2, ci % 2
                nc.gpsimd.dma_start(
                    kt_sb[:, ci],
                    ktag_out[
                        r * D : (r + 1) * D, half * 512 : (half + 1) * 512
                    ].rearrange("(c p) k -> p c k", p=P),
                )

        # ---- V + Q projections ------------------------------------------
        with tc.tile_pool(name="qph", bufs=1) as qpool:
            if version == 5:
                vstg = qpool.tile([P, KH // P, D], BF16)

            vdst = vstg if version == 5 else v_sb
            for st_i in range(KH // P):
                for oh in range(2):
                    ps = pspool.tile([P, 512], F32, tag="mm", bufs=3)
                    for d in range(NDC):
                        nc.tensor.matmul(
                            ps[:],
                            xt_sb[:, st_i // 4, d, (st_i % 4) * P : (st_i % 4 + 1) * P],
                            wv_sb[:, oh, d, :],
                            start=(d == 0),
                            stop=(d == NDC - 1),
                        )
                    copy_out(vdst[:, st_i, oh * 512 : (oh + 1) * 512], ps[:])
                if version == 5 and st_i == 3:
                    nc.sync.dma_start(
                        vag_in[0:512, :].rearrange("(t p) o -> p t o", p=P),
                        vstg[:, 0:4, :],
                    )
                    nc.gpsimd.collective_compute(
                        "AllGather", mybir.AluOpType.bypass, replica_groups=cc,
                        ins=[vag_in[0:512, :]], outs=[vag_out[0:1024, :]],
                    )
                    # gather A yields global s-tiles 0-3 (rank0) + 8-11
                    nc.gpsimd.dma_start(
                        v_sb[:, 0:4, :],
                        vag_out[0:512, :].rearrange("(t p) o -> p t o", p=P),
                    )
                    nc.gpsimd.dma_start(
                        v_sb[:, 8:12, :],
                        vag_out[512:1024, :].rearrange("(t p) o -> p t o", p=P),
                    )
            if version == 5:
                nc.sync.dma_start(
                    vag_in[512:1024, :].rearrange("(t p) o -> p t o", p=P),
                    vstg[:, 4:8, :],
                )
                nc.gpsimd.collective_compute(
                    "AllGather", mybir.AluOpType.bypass, replica_groups=cc,
                    ins=[vag_in[512:1024, :]], outs=[vag_out[1024:2048, :]],
                )
                nc.gpsimd.dma_start(
                    v_sb[:, 4:8, :],
                    vag_out[1024:1536, :].rearrange("(t p) o -> p t o", p=P),
                )
                nc.gpsimd.dma_start(
                    v_sb[:, 12:16, :],
                    vag_out[1536:2048, :].rearrange("(t p) o -> p t o", p=P),
                )

            # Q projection: QT[o, q]
            for qs in range(QCORE // 512):
                for c in range(NDC):
                    ps = pspool.tile([P, 512], F32, tag="mm", bufs=3)
                    for d in range(NDC):
                        nc.tensor.matmul(
                            ps[:],
                            wq_sb[:, c // 4, d, (c % 4) * P : (c % 4 + 1) * P],
                            xtq_sb[:, qs, d, :],
                            start=(d == 0),
                            stop=(d == NDC - 1),
                        )
                    copy_out(qt_sb[:, c, qs * 512 : (qs + 1) * 512], ps[:])

    # ---- attention: S^T-direct over two quads ----------------------------
    with tc.tile_pool(name="ptq", bufs=1) as ptpool, tc.tile_pool(
        name="stats", bufs=4
    ) as stp, tc.tile_pool(name="cp", bufs=2) as cp, tc.tile_pool(
        name="accp", bufs=1
    ) as accp:
        onesb = accp.tile([P, 1], F32)
        maskTsb = accp.tile([P, NQT, 2, P], F32)
        nc.scalar.dma_start(onesb[:], ones[:])
        nc.scalar.dma_start(maskTsb[:], masksT[:])
        pt = {
            g: ptpool.tile(
                [P, 8 * g + 8, 512], BF16, tag=f"ptq{g}", name=f"ptq{g}"
            )
            for g in (0, 1)
        }
        # per-quad f32 rowsum accumulators [k-part, member q cols]; the
        # partial-width adds ([qoff:512]) accumulate exactly each
        # member's causal kb range
        acc = {
            g: accp.tile([P, 512], F32, tag=f"acc{g}", name=f"acc{g}")
            for g in (0, 1)
        }

        def emit_scores_st(g, kbs):
            for kb in kbs:
                p_lo = max(4 * g, kb // 2)
                qoff = (p_lo - 4 * g) * P
                st = pspool.tile([P, 512], F32, tag="mm", bufs=3)
                for c in range(NDC):
                    nc.tensor.matmul(
                        st[:, qoff:512],
                        kt_sb[:, kb // 4, c, (kb % 4) * P : (kb % 4 + 1) * P],
                        qt_sb[:, c, g * 512 + qoff : (g + 1) * 512],
                        start=(c == 0),
                        stop=(c == NDC - 1),
                    )
                pm = kb // 2  # member whose diagonal/overhang region this is
                if pm >= 4 * g:
                    mo = (pm - 4 * g) * P
                    nc.vector.tensor_add(
                        st[:, mo : mo + P], st[:, mo : mo + P],
                        maskTsb[:, pm, kb % 2, :],
                    )
                nc.scalar.activation(
                    pt[g][:, kb, qoff:512],
                    st[:, qoff:512],
                    mybir.ActivationFunctionType.Exp,
                    scale=1.0,
                )
                if kb == 0:
                    nc.vector.tensor_copy(acc[g][:, 0:512], pt[g][:, 0, 0:512])
                else:
                    nc.vector.tensor_add(
                        acc[g][:, qoff:512], acc[g][:, qoff:512],
                        pt[g][:, kb, qoff:512],
                    )

        def emit_pv(g, members):
            for p in members:
                qo = (p - 4 * g) * P
                co0 = pspool.tile([P, 512], F32, tag="co", bufs=4, name=f"co0_{p}")
                co1 = pspool.tile([P, 512], F32, tag="co", bufs=4, name=f"co1_{p}")
                last = 2 * p + 1
                for kb in range(last + 1):
                    lhsT = pt[g][:, kb, qo : qo + P]
                    nc.tensor.matmul(co0[:], lhsT, v_sb[:, kb, 0:512],
                                     start=(kb == 0), stop=(kb == last))
                    nc.tensor.matmul(co1[:], lhsT, v_sb[:, kb, 512:1024],
                                     start=(kb == 0), stop=(kb == last))
                # cross-partition rowsum: ones-contraction of the f32
                # accumulator chunk (q lands on partitions)
                rs = pspool.tile([P, 1], F32, tag="rs", bufs=1, name=f"rs{p}")
                nc.tensor.matmul(rs[:], acc[g][:, qo : qo + P], onesb[:],
                                 start=True, stop=True)
                rcp = stp.tile([P, 1], F32, tag="rcp", name=f"rcp{p}")
                nc.vector.reciprocal(rcp[:], rs[:])
                csb = cp.tile([P, D], F32, tag="csb", name=f"csb{p}")
                for ch in range(4):
                    lo, hi = ch * 256, (ch + 1) * 256
                    src = co0 if ch < 2 else co1
                    slo, shi = (lo, hi) if ch < 2 else (lo - 512, hi - 512)
                    if ch % 2 == 0:
                        nc.vector.tensor_scalar_mul(
                            csb[:, lo:hi], src[:, slo:shi], rcp[:]
                        )
                    else:
                        nc.scalar.activation(
                            csb[:, lo:hi], src[:, slo:shi],
                            mybir.ActivationFunctionType.Copy, scale=rcp[:],
                        )
                    nc.sync.dma_start(
                        out[p * P : (p + 1) * P, lo:hi], csb[:, lo:hi]
                    )

        # order: quad0 scores, quad1 scores (first half), quad0 PV m0/m1,
        # quad1 scores (rest), quad0 PV m2/m3, quad1 PV — PE stays dense
        # while the K/V gathers get maximum slack
        emit_scores_st(0, range(8))
        emit_scores_st(1, range(8))
        emit_pv(0, (0, 1))
        emit_scores_st(1, range(8, 16))
        emit_pv(0, (2, 3))
        emit_pv(1, (4, 5, 6, 7))


def _build(version=1):
    nc = bacc.Bacc("TRN2", target_bir_lowering=False, debug=False, num_devices=8)

    kh = S if version in (1, 3, 4) else S // 2

    def packed(name, cols):
        # host-prepacked [128, cols//512, 8, 512] input (SBUF tile layout)
        return nc.dram_tensor(
            name, [P, cols // 512, NDC, 512], BF16, kind="ExternalInput"
        ).ap()

    xT = packed("xT", kh)
    xTq = packed("xTq", QCORE)
    wqT = packed("wqT", D)
    wkT = packed("wkT", D)
    wvT = packed("wvT", D)
    out = nc.dram_tensor("out", [QCORE, D], F32, kind="ExternalOutput").ap()
    if version in (4, 5):
        masksT = nc.dram_tensor(
            "masksT", [P, NQT, 2, P], F32, kind="ExternalInput"
        ).ap()
        ones = nc.dram_tensor("ones", [P, 1], F32, kind="ExternalInput").ap()
        ktag_in = nc.dram_tensor("ktag_in", [D, S // 2], BF16).ap()
        ktag_out = nc.dram_tensor("ktag_out", [2 * D, S // 2], BF16).ap()
        if version == 4:
            xTk = packed("xTk", S // 2)
            aps_head = (xT, xTk, xTq, wqT, wkT, wvT, masksT, ones, out,
                        ktag_in, ktag_out)
        else:
            vag_in = nc.dram_tensor("vag_in", [S // 2, D], BF16).ap()
            vag_out = nc.dram_tensor("vag_out", [S, D], BF16).ap()
            aps_head = (xT, xTq, wqT, wkT, wvT, masksT, ones, out,
                        ktag_in, ktag_out, vag_in, vag_out)
        with tile.TileContext(nc) as tc, ExitStack() as rctx:
            pspool = rctx.enter_context(
                tc.tile_pool(name="ps", bufs=2, space=bass.MemorySpace.PSUM)
            )
            _emit_body_v45(nc, tc, rctx, aps_head + (pspool,), version=version)
        nc.compile()
        return nc

    masks = nc.dram_tensor("masks", [P, NQT, 256], F32, kind="ExternalInput").ap()
    ident = nc.dram_tensor("ident", [P, P], BF16, kind="ExternalInput").ap()
    if version == 2:
        ktag_in = nc.dram_tensor("ktag_in", [D, S // 2], BF16).ap()
        ktag_out = nc.dram_tensor("ktag_out", [2 * D, S // 2], BF16).ap()
        vag_in = nc.dram_tensor("vag_in", [S // 2, D], BF16).ap()
        vag_out = nc.dram_tensor("vag_out", [S, D], BF16).ap()
        extra = (ktag_in, ktag_out, vag_in, vag_out)
        head = (xT,)
    elif version == 3:
        xTk = packed("xTk", S // 2)
        ktag_in = nc.dram_tensor("ktag_in", [D, S // 2], BF16).ap()
        ktag_out = nc.dram_tensor("ktag_out", [2 * D, S // 2], BF16).ap()
        extra = (ktag_in, ktag_out)
        head = (xT, xTk)
    else:
        extra = ()
        head = (xT,)

    with tile.TileContext(nc) as tc, ExitStack() as rctx:
        pspool = rctx.enter_context(
            tc.tile_pool(name="ps", bufs=2, space=bass.MemorySpace.PSUM)
        )
        aps = head + (xTq, wqT, wkT, wvT, masks, ident, out) + extra + (pspool,)
        _emit_body(nc, tc, rctx, aps, version=version)

    nc.compile()
    return nc


def _pack(wT, gw=512):
    # [1024, cols] -> [128, cols//gw, 8, gw], the SBUF tile layout:
    # pk[p, g, d, o] = wT[d*128 + p, g*gw + o]; contiguous per partition
    cols = wT.shape[1]
    w4 = wT.reshape(NDC, P, cols // gw, gw)
    return np.ascontiguousarray(w4.transpose(1, 2, 0, 3))


def _prep_inputs(x, Wk, Wq, Wv, version=1):
    x = np.asarray(x, dtype=np.float32)
    wqT = _pack((np.asarray(Wq, np.float32).T / 32.0).astype(NPBF16))
    wkT = _pack(np.asarray(Wk, np.float32).T.astype(NPBF16))
    wvT = _pack(np.asarray(Wv, np.float32).T.astype(NPBF16))
    ident = np.eye(P, dtype=NPBF16)

    mask_by_h = {}
    maskT_by_h = {}
    for h in (0, 1):
        mk = np.empty((P, NQT, 256), np.float32)
        for j, t in enumerate(TILES[h]):
            base = LJS[j] - 256
            col = base + np.arange(256)[None, :]
            row = t * P + np.arange(P)[:, None]
            mk[:, j, :] = np.where(col <= row, 0.0, -1e30)
        mask_by_h[h] = mk
        # transposed masks for v4/v5: maskT[kp, p, parity, qc] for
        # k-block kb = 2p + parity vs q-tile t = TILES[h][p]:
        # allowed iff kb*128 + kp <= t*128 + qc
        mt = np.empty((P, NQT, 2, P), np.float32)
        kp = np.arange(P)[:, None]
        qc = np.arange(P)[None, :]
        for p, t in enumerate(TILES[h]):
            for parity in (0, 1):
                kb = 2 * p + parity
                mt[:, p, parity, :] = np.where(
                    kb * P + kp <= t * P + qc, 0.0, -1e30
                )
        maskT_by_h[h] = mt

    in_maps = []
    for c in range(8):
        b, h = c // 2, c % 2
        xTb = np.ascontiguousarray(x[b].T.astype(NPBF16))
        qcols = np.concatenate([np.arange(t * P, (t + 1) * P) for t in TILES[h]])
        xt_in = (
            xTb
            if version in (1, 3, 4)
            else xTb[:, h * (S // 2) : (h + 1) * (S // 2)]
        )
        m = {
            "xT": _pack(xt_in),
            "xTq": _pack(xTb[:, qcols]),
            "wqT": wqT,
            "wkT": wkT,
            "wvT": wvT,
        }
        if version in (4, 5):
            m["masksT"] = maskT_by_h[h]
            m["ones"] = np.ones((P, 1), np.float32)
        else:
            m["masks"] = mask_by_h[h]
            m["ident"] = ident
        if version in (3, 4):
            m["xTk"] = _pack(xTb[:, h * (S // 2) : (h + 1) * (S // 2)])
        in_maps.append(m)
    return in_maps


VERSION = int(os.environ.get("BASS_KERNEL_VERSION", "3"))


def kernel(x, Wk, Wq, Wv):
    global LAST_RESULTS
    if VERSION not in _COMPILED:
        _COMPILED[VERSION] = _build(VERSION)
    nc = _COMPILED[VERSION]
    in_maps = _prep_inputs(x, Wk, Wq, Wv, version=VERSION)
    trace = bool(int(os.environ.get("BASS_KERNEL_TRACE", "0")))
    res = run_bass_kernel_spmd(nc, in_maps, list(range(8)), trace=trace)
    LAST_RESULTS = res
    out = np.empty((B, S, D), np.float32)
    for c in range(8):
        b, h = c // 2, c % 2
        oc = res.results[c]["out"]
        for j, t in enumerate(TILES[h]):
            out[b, t * P : (t + 1) * P, :] = oc[j * P : (j + 1) * P, :]
    return out



# revision 18
# speedup vs baseline: 1.0184x; 1.0184x over previous
"""Causal attention (B=4, S=2048, D=1024) on 8 trn2 NeuronCores.

Sharding: core c = (batch b = c//2, query-group h = c%2). Default scheme
(version 3): each core K-projects its OWN key half (pairwise AllGather of KT
hides behind the V projection), V-projects its whole batch locally, and
Q-projects its own 8 query tiles of 128 rows. Tiles are interleaved (t % 4
in {0,3} for h=0, {1,2} for h=1) so both cores of a pair have the same
causal work profile and the SPMD program is identical on every core.

All matmul operands are bf16 (fp32 PSUM accumulation): halves DMA bytes and
SBUF footprint vs f32r, so x / K / V / Q all stay SBUF-resident. Inputs are
host-prepacked into the exact SBUF tile layout [128, G, 8, 512] so every
load is one DMA with 128 contiguous per-partition runs (DIRECT2D descriptor
generation is serialized per sequencer and costs ~5ns/descriptor — layout,
batching, and spreading issuance across the sync/scalar/gpsimd queues keep
it off the critical path). Collective-dependent readbacks ride the gpsimd
queue: a sync-queue wait on an unfinished collective deadlocks.

Device kernel per core:
  KT[o,k] = sum_d WkT[d,o] xTk[d,k]         k = own 1024 keys, then
                                            pairwise AllGather -> all 2048
  V[s,o]  = sum_d xT[d,s]  WvT[d,o]         s = 0..2047 (local, duplicated)
  QT[o,q] = sum_d WqT[d,o] xTq[d,q]         q = core's 1024 rows
                                            (Wq pre-scaled by 1/32 on host)
  per sorted q-tile position j (L = (2j+2)*128 keys, both h fit under L):
    S[q,k] = sum_o QT[o,q] KT[o,k];  last 256 cols += mask (covers diag
             block + the 128-col overhang the other h-core doesn't need)
    P = exp(S)  (no rowmax subtraction: |S| <= ~6, exp is fp32-safe;
             masked cols are -1e30 -> exp underflows to exactly 0)
    rowsum fused via activation accum_out
    C[q,:] = sum_k P^T[k,q] V[k,:]  (P^T via PE transpose, bf16)
    out = C * (1/rowsum)
"""

import os
import sys
from contextlib import ExitStack

import ml_dtypes
import numpy as np

sys.path.insert(0, "/opt/trn_rl_repo")

import concourse.bass as bass
import concourse.tile as tile
from concourse import bacc, mybir
from concourse.bass_utils import run_bass_kernel_spmd

F32 = mybir.dt.float32
BF16 = mybir.dt.bfloat16
NPBF16 = ml_dtypes.bfloat16
P = 128
B, S, D = 4, 2048, 1024
NDC = D // P                     # 8 contraction chunks of 128
NQT = 8                          # q-tiles of 128 rows per core
QCORE = NQT * P                  # 1024 q rows per core
TILES = {
    0: [t for t in range(16) if t % 4 in (0, 3)],
    1: [t for t in range(16) if t % 4 in (1, 2)],
}
# position j covers L_j = (2j+2)*128 key columns: the max over the two
# h-cores' causal needs at that sorted position; the mask input zeroes the
# per-core overhang (at most 128 cols, always inside the last 256).
LJS = [(2 * j + 2) * P for j in range(NQT)]

_COMPILED = {}
LAST_RESULTS = None


def _score_chunks(L):
    """Split L key cols into matmul chunks <=512; last chunk is the 256-wide
    mask window."""
    pre = L - 256
    chunks = []
    off = 0
    while pre - off >= 512:
        chunks.append((off, 512, False))
        off += 512
    if pre - off:
        chunks.append((off, pre - off, False))
    chunks.append((pre, 256, True))
    return chunks


def _emit_body(nc, tc, rctx, aps, version=1):
    if version == 1:
        xT, xTq, wqT, wkT, wvT, masks, ident, out, pspool = aps
        cc = None
    elif version == 2:
        (xT, xTq, wqT, wkT, wvT, masks, ident, out,
         ktag_in, ktag_out, vag_in, vag_out, pspool) = aps
        cc = [[0, 1], [2, 3], [4, 5], [6, 7]]
    else:  # version 3: K gathered pairwise, V+Q local
        (xT, xTk, xTq, wqT, wkT, wvT, masks, ident, out,
         ktag_in, ktag_out, pspool) = aps
        cc = [[0, 1], [2, 3], [4, 5], [6, 7]]
    KH = S if version in (1, 3) else S // 2  # value rows projected locally
    KK = S // 2 if version in (2, 3) else S  # key rows projected locally
    copy_ctr = [0]

    def copy_out(dst, src):
        # alternate PSUM->SBUF copies between vector and scalar engines
        copy_ctr[0] += 1
        if copy_ctr[0] % 2:
            nc.vector.tensor_copy(dst, src)
        else:
            nc.scalar.copy(dst, src)

    cpool = rctx.enter_context(tc.tile_pool(name="const", bufs=1))
    identsb = cpool.tile([P, P], BF16)
    masksb = cpool.tile([P, NQT, 256], F32)
    ktpool = rctx.enter_context(tc.tile_pool(name="ktp", bufs=1))
    kt_sb = ktpool.tile([P, NDC, S], BF16)     # KT: [o%128, o//128, k]
    vpool = rctx.enter_context(tc.tile_pool(name="vp", bufs=1))
    v_sb = vpool.tile([P, S // P, D], BF16)    # V: [s%128, s//128, o]
    qtpool = rctx.enter_context(tc.tile_pool(name="qtp", bufs=1))
    qt_sb = qtpool.tile([P, NDC, QCORE], BF16)  # QT: [o%128, o//128, q]

    with tc.tile_pool(name="wts", bufs=1) as wpool:
        wv_sb = wpool.tile([P, 2, NDC, 512], BF16)
        wq_sb = wpool.tile([P, 2, NDC, 512], BF16)
        xt_sb = wpool.tile([P, KH // 512, NDC, 512], BF16)  # [p, s//512, d, s%512]

        # inputs are host-prepacked as [128, G, 8, 512] (exact SBUF tile
        # layout, contiguous per partition): each 512-col group is one DMA
        # with 128 contiguous runs -> cheap descriptor generation
        def ld(eng, dst, w, g):
            eng.dma_start(dst[:, g], w[:, g])

        # ---- K phase: wk/xtk/kstg live only here (SBUF headroom) ---------
        with tc.tile_pool(name="kph", bufs=1) as kpool:
            wk_sb = kpool.tile([P, 2, NDC, 512], BF16)
            if version == 3:
                xtk_sb = kpool.tile([P, KK // 512, NDC, 512], BF16)  # own-half xT
            else:
                xtk_sb = xt_sb

            # issue order = first-use order; spread across the sync /
            # scalar / vector HWDGE queues so descriptor generation
            # doesn't serialize behind one sequencer
            xk_src = xTk if version == 3 else xT
            ld(nc.sync, wk_sb, wkT, 0)
            ld(nc.sync, xtk_sb, xk_src, 0)
            ld(nc.sync, wk_sb, wkT, 1)
            for g in range(1, KK // 512):
                ld(nc.sync, xtk_sb, xk_src, g)
            if version == 3:
                for g in range(KH // 512):
                    ld(nc.scalar, xt_sb, xT, g)
            ld(nc.sync, wv_sb, wvT, 0)
            ld(nc.sync, wv_sb, wvT, 1)
            ld(nc.scalar, wq_sb, wqT, 0)
            ld(nc.scalar, wq_sb, wqT, 1)
            nc.scalar.dma_start(identsb[:], ident[:])
            nc.scalar.dma_start(masksb[:], masks[:])

            # ---- K projection: KT[o, own keys] ---------------------------
            for ks in range(KK // 512):
                if version in (2, 3):
                    kst = kpool.tile([P, NDC, 512], BF16, tag="kstg", bufs=2)
                for c in range(NDC):
                    ps = pspool.tile([P, 512], F32, tag="mm", bufs=3)
                    for d in range(NDC):
                        nc.tensor.matmul(
                            ps[:],
                            wk_sb[:, c // 4, d, (c % 4) * P : (c % 4 + 1) * P],
                            xtk_sb[:, ks, d, :],
                            start=(d == 0),
                            stop=(d == NDC - 1),
                        )
                    if version == 1:
                        copy_out(kt_sb[:, c, ks * 512 : (ks + 1) * 512], ps[:])
                    else:
                        copy_out(kst[:, c, :], ps[:])
                if version in (2, 3):
                    # stage this k-chunk to DRAM for the pairwise gather
                    nc.gpsimd.dma_start(
                        ktag_in[:, ks * 512 : (ks + 1) * 512].rearrange(
                            "(c p) k -> p c k", p=P
                        ),
                        kst[:],
                    )
            if version in (2, 3):
                nc.gpsimd.collective_compute(
                    "AllGather", mybir.AluOpType.bypass, replica_groups=cc,
                    ins=[ktag_in[:]], outs=[ktag_out[:]],
                )
            if version == 3:
                # kt readback on the idle gpsimd queue, right behind the
                # collective: no sync-queue sem wait, so V/Q-proj DMA waits
                # can't get serialized behind the gather
                for r in range(2):
                    nc.gpsimd.dma_start(
                        kt_sb[:, :, r * KK : (r + 1) * KK],
                        ktag_out[r * D : (r + 1) * D, :].rearrange(
                            "(c p) k -> p c k", p=P
                        ),
                    )

        # ---- post-K pool: xtq (+ v2 staging) in the freed space ----------
        with tc.tile_pool(name="qph", bufs=1) as qpool:
            xtq_sb = qpool.tile([P, 2, NDC, 512], BF16)
            if version == 2:
                vstg = qpool.tile([P, KH // P, D], BF16)  # staged V
            ld(nc.scalar, xtq_sb, xTq, 0)
            ld(nc.scalar, xtq_sb, xTq, 1)

            # ---- V projection: V[own rows, o] --------------------------------
            vdst = vstg if version == 2 else v_sb
            for st_i in range(KH // P):
                for oh in range(2):
                    ps = pspool.tile([P, 512], F32, tag="mm", bufs=3)
                    for d in range(NDC):
                        nc.tensor.matmul(
                            ps[:],
                            xt_sb[:, st_i // 4, d, (st_i % 4) * P : (st_i % 4 + 1) * P],
                            wv_sb[:, oh, d, :],
                            start=(d == 0),
                            stop=(d == NDC - 1),
                        )
                    copy_out(vdst[:, st_i, oh * 512 : (oh + 1) * 512], ps[:])
                if version == 2 and st_i == 3:
                    # first V half staged -> gather it while the second half
                    # computes; kt readback rides the gpsimd queue in between
                    # (K gather already done, so it doesn't block the V gather).
                    # Gather A rows = [s 0:512 | s 1024:1536] (rank-major).
                    nc.sync.dma_start(
                        vag_in[0:512, :].rearrange("(t p) o -> p t o", p=P),
                        vstg[:, 0:4, :],
                    )
                    nc.gpsimd.collective_compute(
                        "AllGather", mybir.AluOpType.bypass, replica_groups=cc,
                        ins=[vag_in[0:512, :]], outs=[vag_out[0:1024, :]],
                    )
                    # kt readback on gpsimd behind gather A (K gather long
                    # done); keeping collective-dependent waits off the sync
                    # queue — a sync wait on an unfinished collective deadlocks
                    for r in range(2):
                        nc.gpsimd.dma_start(
                            kt_sb[:, :, r * KH : (r + 1) * KH],
                            ktag_out[r * D : (r + 1) * D, :].rearrange(
                                "(c p) k -> p c k", p=P
                            ),
                        )
                    # vagA available: v s-tiles 0-3 and 8-11
                    nc.gpsimd.dma_start(
                        v_sb[:, 0:4, :],
                        vag_out[0:512, :].rearrange("(t p) o -> p t o", p=P),
                    )
                    nc.gpsimd.dma_start(
                        v_sb[:, 8:12, :],
                        vag_out[512:1024, :].rearrange("(t p) o -> p t o", p=P),
                    )
            if version == 2:
                # Gather B rows = [s 512:1024 | s 1536:2048]
                nc.sync.dma_start(
                    vag_in[512:1024, :].rearrange("(t p) o -> p t o", p=P),
                    vstg[:, 4:8, :],
                )
                nc.gpsimd.collective_compute(
                    "AllGather", mybir.AluOpType.bypass, replica_groups=cc,
                    ins=[vag_in[512:1024, :]], outs=[vag_out[1024:2048, :]],
                )

            # ---- Q projection: QT[o, q] --------------------------------------
            for qs in range(QCORE // 512):
                for c in range(NDC):
                    ps = pspool.tile([P, 512], F32, tag="mm", bufs=3)
                    for d in range(NDC):
                        nc.tensor.matmul(
                            ps[:],
                            wq_sb[:, c // 4, d, (c % 4) * P : (c % 4 + 1) * P],
                            xtq_sb[:, qs, d, :],
                            start=(d == 0),
                            stop=(d == NDC - 1),
                        )
                    copy_out(qt_sb[:, c, qs * 512 : (qs + 1) * 512], ps[:])

            if version == 2:
                # vagB readback: v s-tiles 4-7 first (needed by PV from j=2 on)
                nc.gpsimd.dma_start(
                    v_sb[:, 4:8, :],
                    vag_out[1024:1536, :].rearrange("(t p) o -> p t o", p=P),
                )
                nc.gpsimd.dma_start(
                    v_sb[:, 12:16, :],
                    vag_out[1536:2048, :].rearrange("(t p) o -> p t o", p=P),
                )

    # ---- attention, software-pipelined per q-tile position ---------------
    with tc.tile_pool(name="sp", bufs=2) as sp, tc.tile_pool(
        name="pp", bufs=2
    ) as pp, tc.tile_pool(name="stats", bufs=4) as stp, tc.tile_pool(
        name="atp", bufs=4
    ) as atp, tc.tile_pool(name="cp", bufs=2) as cp:
        state = {}

        def emit_scores(j):
            L = LJS[j]
            ssb = sp.tile([P, L], F32, tag="ssb", name=f"ssb{j}")
            for off, w, is_mask in _score_chunks(L):
                ps = pspool.tile([P, 512], F32, tag="mm", bufs=3)
                for c in range(NDC):
                    nc.tensor.matmul(
                        ps[:, :w],
                        qt_sb[:, c, j * P : (j + 1) * P],
                        kt_sb[:, c, off : off + w],
                        start=(c == 0),
                        stop=(c == NDC - 1),
                    )
                if is_mask:
                    nc.vector.tensor_add(
                        ssb[:, off : off + w], ps[:, :w], masksb[:, j, :]
                    )
                else:
                    copy_out(ssb[:, off : off + w], ps[:, :w])
            state[j] = ssb

        def emit_softmax_pv(j):
            L = LJS[j]
            ssb = state.pop(j)
            psb = pp.tile([P, L], BF16, tag="psb", name=f"psb{j}")
            sumv = stp.tile([P, 1], F32, tag="sumv", name=f"sumv{j}")
            nc.scalar.activation(
                psb[:],
                ssb[:],
                mybir.ActivationFunctionType.Exp,
                scale=1.0,
                accum_out=sumv[:],
            )
            rcp = stp.tile([P, 1], F32, tag="rcp", name=f"rcp{j}")
            nc.vector.reciprocal(rcp[:], sumv[:])

            co0 = pspool.tile([P, 512], F32, tag="co", bufs=2, name=f"co0_{j}")
            co1 = pspool.tile([P, 512], F32, tag="co", bufs=2, name=f"co1_{j}")
            nkt = L // P
            for k in range(nkt):
                tp = pspool.tile([P, P], BF16, tag="tp", bufs=3, name=f"tp{j}_{k}")
                nc.tensor.transpose(tp[:], psb[:, k * P : (k + 1) * P], identsb[:])
                at = atp.tile([P, P], BF16, tag="at", name=f"at{j}_{k}")
                copy_out(at[:], tp[:])
                nc.tensor.matmul(
                    co0[:], at[:], v_sb[:, k, 0:512],
                    start=(k == 0), stop=(k == nkt - 1),
                )
                nc.tensor.matmul(
                    co1[:], at[:], v_sb[:, k, 512:1024],
                    start=(k == 0), stop=(k == nkt - 1),
                )
            csb = cp.tile([P, D], F32, tag="csb", name=f"csb{j}")
            nc.vector.tensor_scalar_mul(csb[:, 0:512], co0[:], rcp[:])
            nc.sync.dma_start(out[j * P : (j + 1) * P, 0:512], csb[:, 0:512])
            nc.scalar.activation(
                csb[:, 512:1024],
                co1[:],
                mybir.ActivationFunctionType.Copy,
                scale=rcp[:],
            )
            nc.sync.dma_start(out[j * P : (j + 1) * P, 512:1024], csb[:, 512:1024])

        emit_scores(0)
        for j in range(1, NQT):
            emit_scores(j)
            emit_softmax_pv(j - 1)
        emit_softmax_pv(NQT - 1)


def _emit_body_v45(nc, tc, rctx, aps, version=4):
    """S^T-direct attention (no PE transposes). version 4: V projected
    locally for all 2048 rows (duplicated per pair, like v3). version 5:
    V projection sharded by rows + pairwise AllGather (like v2).

    Attention works on two quads of 4 sorted q-tile positions each. For
    quad g (positions p = 4g..4g+3), scores are computed TRANSPOSED:
      S^T[k, q] = sum_o KT[o, k-block] QT[o, q]   (k on partitions)
    per 128-row k-block kb, over the q columns of all members that
    causally need kb (member p needs kb <= 2p+1 across both h cores; the
    h-specific overhang + diagonal triangle are handled by a per-core
    maskT input added on DVE). exp(S^T) lands directly in the PV
    stationary layout, so the 72 PE transposes + PSUM->SBUF P^T copies
    of v3 disappear. The softmax row-sum rides a 1-column ones-matmul
    that reuses the already-loaded P^T stationary.
    """
    if version == 4:
        (xT, xTk, xTq, wqT, wkT, wvT, masksT, ones, out,
         ktag_in, ktag_out, pspool) = aps
    else:
        (xT, xTq, wqT, wkT, wvT, masksT, ones, out,
         ktag_in, ktag_out, vag_in, vag_out, pspool) = aps
    cc = [[0, 1], [2, 3], [4, 5], [6, 7]]
    KH = S if version == 4 else S // 2   # V rows projected locally
    KK = S // 2                          # K rows projected locally
    copy_ctr = [0]

    def copy_out(dst, src):
        copy_ctr[0] += 1
        if copy_ctr[0] % 2:
            nc.vector.tensor_copy(dst, src)
        else:
            nc.scalar.copy(dst, src)

    ktpool = rctx.enter_context(tc.tile_pool(name="ktp", bufs=1))
    # KT chunk-major: [o%128, key-chunk(512), o//128, k%512] so each
    # gather-readback DMA writes one contiguous chunk (no interleaved
    # address ranges -> no false deps stalling the first scores)
    kt_sb = ktpool.tile([P, 4, NDC, 512], BF16)
    vpool = rctx.enter_context(tc.tile_pool(name="vp", bufs=1))
    v_sb = vpool.tile([P, S // P, D], BF16)    # V: [s%128, s//128, o]
    qtpool = rctx.enter_context(tc.tile_pool(name="qtp", bufs=1))
    qt_sb = qtpool.tile([P, NDC, QCORE], BF16)  # QT: [o%128, o//128, q]

    with tc.tile_pool(name="wts", bufs=1) as wpool:
        wv_sb = wpool.tile([P, 2, NDC, 512], BF16)
        wq_sb = wpool.tile([P, 2, NDC, 512], BF16)
        xtq_sb = wpool.tile([P, 2, NDC, 512], BF16)
        xt_sb = wpool.tile([P, KH // 512, NDC, 512], BF16)

        def ld(eng, dst, w, g):
            eng.dma_start(dst[:, g], w[:, g])

        def ld2(eng_a, eng_b, dst, w, g):
            # split one 512-col group across two DMA queues (halves the
            # startup-critical load latency); only sync/scalar/gpsimd
            # have DMA queues
            eng_a.dma_start(dst[:, g, 0:4], w[:, g, 0:4])
            eng_b.dma_start(dst[:, g, 4:8], w[:, g, 4:8])

        # ---- K phase ----------------------------------------------------
        with tc.tile_pool(name="kph", bufs=1) as kpool:
            wk_sb = kpool.tile([P, 2, NDC, 512], BF16)
            if version == 4:
                xtk_sb = kpool.tile([P, KK // 512, NDC, 512], BF16)
                xk_src = xTk
            else:
                xtk_sb = xt_sb
                xk_src = xT

            # The 3 DMA queues each sustain only ~90GB/s and sit idle
            # for the first ~11us, and the kernel needs 14MB of input
            # streamed just-in-time. Emit ALL loads as 256KB 2-d-chunk
            # pieces, round-robin across queues in exact first-use
            # order. gpsimd takes only pieces needed before ~t=24us —
            # after that it is reserved for K staging + collective +
            # readbacks.
            def ldp(eng, dst, w, g, d0, d1):
                eng.dma_start(dst[:, g, d0:d1], w[:, g, d0:d1])

            early = [nc.sync, nc.scalar, nc.gpsimd]
            late = [nc.sync, nc.scalar]
            # (dst, src, group) in first-use order
            k_loads = [(wk_sb, wkT, 0), (xtk_sb, xk_src, 0),
                       (wk_sb, wkT, 1)]
            if version == 4:
                k_loads.append((xtk_sb, xTk, 1))
                v_loads = [(wv_sb, wvT, 0), (xt_sb, xT, 0),
                           (wv_sb, wvT, 1), (xt_sb, xT, 1),
                           (xt_sb, xT, 2), (xt_sb, xT, 3)]
            else:
                k_loads.append((xt_sb, xT, 1))
                v_loads = [(wv_sb, wvT, 0), (wv_sb, wvT, 1)]
            q_loads = [(wq_sb, wqT, 0), (xtq_sb, xTq, 0),
                       (wq_sb, wqT, 1), (xtq_sb, xTq, 1)]
            i = 0
            for dst, w, g in k_loads:
                for dd in range(4):
                    ldp(early[i % 3], dst, w, g, 2 * dd, 2 * dd + 2)
                    i += 1
            i = 0
            for dst, w, g in v_loads + q_loads:
                for dd in range(4):
                    ldp(late[i % 2], dst, w, g, 2 * dd, 2 * dd + 2)
                    i += 1

            # K projection: KT[o, own keys]; stage per 4-c half (512KB
            # DMAs: descriptor generation on the gpsimd engine costs
            # ~1us per DMA, so fewer+bigger beats 16x128KB) with a
            # 3-buffered half-size staging tile so the next group's
            # copy-outs never wait on a staging drain
            for ks in range(KK // 512):
                for ch in range(2):
                    kst = kpool.tile(
                        [P, 4, 512], BF16, tag="kstg", bufs=3,
                        name=f"kst{ks}_{ch}",
                    )
                    for c4 in range(4):
                        c = ch * 4 + c4
                        ps = pspool.tile([P, 512], F32, tag="mm", bufs=3)
                        for d in range(NDC):
                            nc.tensor.matmul(
                                ps[:],
                                wk_sb[:, c // 4, d, (c % 4) * P : (c % 4 + 1) * P],
                                xtk_sb[:, ks, d, :],
                                start=(d == 0),
                                stop=(d == NDC - 1),
                            )
                        copy_out(kst[:, c4, :], ps[:])
                    nc.gpsimd.dma_start(
                        ktag_in[
                            ch * 512 : (ch + 1) * 512,
                            ks * 512 : (ks + 1) * 512,
                        ].rearrange("(c p) k -> p c k", p=P),
                        kst[:],
                    )
            nc.gpsimd.collective_compute(
                "AllGather", mybir.AluOpType.bypass, replica_groups=cc,
                ins=[ktag_in[:]], outs=[ktag_out[:]],
            )
            # readback in 4 chunked DMAs; chunk ci = global keys
            # ci*512:(ci+1)*512 = ktag_out[rank(ci)*D rows, (ci%2) cols]
            for ci in range(4):
                r, half = ci // 2, ci % 2
                nc.gpsimd.dma_start(
                    kt_sb[:, ci],
                    ktag_out[
                        r * D : (r + 1) * D, half * 512 : (half + 1) * 512
                    ].rearrange("(c p) k -> p c k", p=P),
                )

        # ---- V + Q projections ------------------------------------------
        with tc.tile_pool(name="qph", bufs=1) as qpool:
            if version == 5:
                vstg = qpool.tile([P, KH // P, D], BF16)

            vdst = vstg if version == 5 else v_sb
            for st_i in range(KH // P):
                for oh in range(2):
                    ps = pspool.tile([P, 512], F32, tag="mm", bufs=3)
                    for d in range(NDC):
                        nc.tensor.matmul(
                            ps[:],
                            xt_sb[:, st_i // 4, d, (st_i % 4) * P : (st_i % 4 + 1) * P],
                            wv_sb[:, oh, d, :],
                            start=(d == 0),
                            stop=(d == NDC - 1),
                        )
                    copy_out(vdst[:, st_i, oh * 512 : (oh + 1) * 512], ps[:])
                if version == 5 and st_i == 3:
                    nc.sync.dma_start(
                        vag_in[0:512, :].rearrange("(t p) o -> p t o", p=P),
                        vstg[:, 0:4, :],
                    )
                    nc.gpsimd.collective_compute(
                        "AllGather", mybir.AluOpType.bypass, replica_groups=cc,
                        ins=[vag_in[0:512, :]], outs=[vag_out[0:1024, :]],
                    )
                    # gather A yields global s-tiles 0-3 (rank0) + 8-11
                    nc.gpsimd.dma_start(
                        v_sb[:, 0:4, :],
                        vag_out[0:512, :].rearrange("(t p) o -> p t o", p=P),
                    )
                    nc.gpsimd.dma_start(
                        v_sb[:, 8:12, :],
                        vag_out[512:1024, :].rearrange("(t p) o -> p t o", p=P),
                    )
            if version == 5:
                nc.sync.dma_start(
                    vag_in[512:1024, :].rearrange("(t p) o -> p t o", p=P),
                    vstg[:, 4:8, :],
                )
                nc.gpsimd.collective_compute(
                    "AllGather", mybir.AluOpType.bypass, replica_groups=cc,
                    ins=[vag_in[512:1024, :]], outs=[vag_out[1024:2048, :]],
                )
                nc.gpsimd.dma_start(
                    v_sb[:, 4:8, :],
                    vag_out[1024:1536, :].rearrange("(t p) o -> p t o", p=P),
                )
                nc.gpsimd.dma_start(
                    v_sb[:, 12:16, :],
                    vag_out[1536:2048, :].rearrange("(t p) o -> p t o", p=P),
                )

            # Q projection: QT[o, q]
            for qs in range(QCORE // 512):
                for c in range(NDC):
                    ps = pspool.tile([P, 512], F32, tag="mm", bufs=3)
                    for d in range(NDC):
                        nc.tensor.matmul(
                            ps[:],
                            wq_sb[:, c // 4, d, (c % 4) * P : (c % 4 + 1) * P],
                            xtq_sb[:, qs, d, :],
                            start=(d == 0),
                            stop=(d == NDC - 1),
                        )
                    copy_out(qt_sb[:, c, qs * 512 : (qs + 1) * 512], ps[:])

    # ---- attention: S^T-direct over two quads ----------------------------
    with tc.tile_pool(name="ptq", bufs=1) as ptpool, tc.tile_pool(
        name="stats", bufs=4
    ) as stp, tc.tile_pool(name="cp", bufs=2) as cp, tc.tile_pool(
        name="accp", bufs=1
    ) as accp:
        onesb = accp.tile([P, 1], F32)
        maskTsb = accp.tile([P, NQT, 2, P], F32)
        nc.scalar.dma_start(onesb[:], ones[:])
        nc.scalar.dma_start(maskTsb[:], masksT[:])
        pt = {
            g: ptpool.tile(
                [P, 8 * g + 8, 512], BF16, tag=f"ptq{g}", name=f"ptq{g}"
            )
            for g in (0, 1)
        }
        # per-quad f32 rowsum accumulators [k-part, member q cols]; the
        # partial-width adds ([qoff:512]) accumulate exactly each
        # member's causal kb range
        acc = {
            g: accp.tile([P, 512], F32, tag=f"acc{g}", name=f"acc{g}")
            for g in (0, 1)
        }

        def emit_scores_st(g, kbs):
            for kb in kbs:
                p_lo = max(4 * g, kb // 2)
                qoff = (p_lo - 4 * g) * P
                st = pspool.tile([P, 512], F32, tag="mm", bufs=3)
                for c in range(NDC):
                    nc.tensor.matmul(
                        st[:, qoff:512],
                        kt_sb[:, kb // 4, c, (kb % 4) * P : (kb % 4 + 1) * P],
                        qt_sb[:, c, g * 512 + qoff : (g + 1) * 512],
                        start=(c == 0),
                        stop=(c == NDC - 1),
                    )
                pm = kb // 2  # member whose diagonal/overhang region this is
                if pm >= 4 * g:
                    mo = (pm - 4 * g) * P
                    nc.vector.tensor_add(
                        st[:, mo : mo + P], st[:, mo : mo + P],
                        maskTsb[:, pm, kb % 2, :],
                    )
                nc.scalar.activation(
                    pt[g][:, kb, qoff:512],
                    st[:, qoff:512],
                    mybir.ActivationFunctionType.Exp,
                    scale=1.0,
                )
                if kb == 0:
                    nc.vector.tensor_copy(acc[g][:, 0:512], pt[g][:, 0, 0:512])
                else:
                    nc.vector.tensor_add(
                        acc[g][:, qoff:512], acc[g][:, qoff:512],
                        pt[g][:, kb, qoff:512],
                    )

        def emit_pv(g, members):
            for p in members:
                qo = (p - 4 * g) * P
                co0 = pspool.tile([P, 512], F32, tag="co", bufs=4, name=f"co0_{p}")
                co1 = pspool.tile([P, 512], F32, tag="co", bufs=4, name=f"co1_{p}")
                last = 2 * p + 1
                for kb in range(last + 1):
                    lhsT = pt[g][:, kb, qo : qo + P]
                    nc.tensor.matmul(co0[:], lhsT, v_sb[:, kb, 0:512],
                                     start=(kb == 0), stop=(kb == last))
                    nc.tensor.matmul(co1[:], lhsT, v_sb[:, kb, 512:1024],
                                     start=(kb == 0), stop=(kb == last))
                # cross-partition rowsum: ones-contraction of the f32
                # accumulator chunk (q lands on partitions)
                rs = pspool.tile([P, 1], F32, tag="rs", bufs=1, name=f"rs{p}")
                nc.tensor.matmul(rs[:], acc[g][:, qo : qo + P], onesb[:],
                                 start=True, stop=True)
                rcp = stp.tile([P, 1], F32, tag="rcp", name=f"rcp{p}")
                nc.vector.reciprocal(rcp[:], rs[:])
                csb = cp.tile([P, D], F32, tag="csb", name=f"csb{p}")
                for ch in range(4):
                    lo, hi = ch * 256, (ch + 1) * 256
                    src = co0 if ch < 2 else co1
                    slo, shi = (lo, hi) if ch < 2 else (lo - 512, hi - 512)
                    if ch % 2 == 0:
                        nc.vector.tensor_scalar_mul(
                            csb[:, lo:hi], src[:, slo:shi], rcp[:]
                        )
                    else:
                        nc.scalar.activation(
                            csb[:, lo:hi], src[:, slo:shi],
                            mybir.ActivationFunctionType.Copy, scale=rcp[:],
                        )
                    nc.sync.dma_start(
                        out[p * P : (p + 1) * P, lo:hi], csb[:, lo:hi]
                    )

        # order: quad0 scores, quad1 scores (first half), quad0 PV m0/m1,
        # quad1 scores (rest), quad0 PV m2/m3, quad1 PV — PE stays dense
        # while the K/V gathers get maximum slack
        emit_scores_st(0, range(8))
        emit_scores_st(1, range(8))
        emit_pv(0, (0, 1))
        emit_scores_st(1, range(8, 16))
        emit_pv(0, (2, 3))
        emit_pv(1, (4, 5, 6, 7))


def _build(version=1):
    nc = bacc.Bacc("TRN2", target_bir_lowering=False, debug=False, num_devices=8)

    kh = S if version in (1, 3, 4) else S // 2

    def packed(name, cols):
        # host-prepacked [128, cols//512, 8, 512] input (SBUF tile layout)
        return nc.dram_tensor(
            name, [P, cols // 512, NDC, 512], BF16, kind="ExternalInput"
        ).ap()

    xT = packed("xT", kh)
    xTq = packed("xTq", QCORE)
    wqT = packed("wqT", D)
    wkT = packed("wkT", D)
    wvT = packed("wvT", D)
    out = nc.dram_tensor("out", [QCORE, D], F32, kind="ExternalOutput").ap()
    if version in (4, 5):
        masksT = nc.dram_tensor(
            "masksT", [P, NQT, 2, P], F32, kind="ExternalInput"
        ).ap()
        ones = nc.dram_tensor("ones", [P, 1], F32, kind="ExternalInput").ap()
        ktag_in = nc.dram_tensor("ktag_in", [D, S // 2], BF16).ap()
        ktag_out = nc.dram_tensor("ktag_out", [2 * D, S // 2], BF16).ap()
        if version == 4:
            xTk = packed("xTk", S // 2)
            aps_head = (xT, xTk, xTq, wqT, wkT, wvT, masksT, ones, out,
                        ktag_in, ktag_out)
        else:
            vag_in = nc.dram_tensor("vag_in", [S // 2, D], BF16).ap()
            vag_out = nc.dram_tensor("vag_out", [S, D], BF16).ap()
            aps_head = (xT, xTq, wqT, wkT, wvT, masksT, ones, out,
                        ktag_in, ktag_out, vag_in, vag_out)
        with tile.TileContext(nc) as tc, ExitStack() as rctx:
            pspool = rctx.enter_context(
                tc.tile_pool(name="ps", bufs=2, space=bass.MemorySpace.PSUM)
            )
            _emit_body_v45(nc, tc, rctx, aps_head + (pspool,), version=version)
        nc.compile()
        return nc

    masks = nc.dram_tensor("masks", [P, NQT, 256], F32, kind="ExternalInput").ap()
    ident = nc.dram_tensor("ident", [P, P], BF16, kind="ExternalInput").ap()
    if version == 2:
        ktag_in = nc.dram_tensor("ktag_in", [D, S // 2], BF16).ap()
        ktag_out = nc.dram_tensor("ktag_out", [2 * D, S // 2], BF16).ap()
        vag_in = nc.dram_tensor("vag_in", [S // 2, D], BF16).ap()
        vag_out = nc.dram_tensor("vag_out", [S, D], BF16).ap()
        extra = (ktag_in, ktag_out, vag_in, vag_out)
        head = (xT,)
    elif version == 3:
        xTk = packed("xTk", S // 2)
        ktag_in = nc.dram_tensor("ktag_in", [D, S // 2], BF16).ap()
        ktag_out = nc.dram_tensor("ktag_out", [2 * D, S // 2], BF16).ap()
        extra = (ktag_in, ktag_out)
        head = (xT, xTk)
    else:
        extra = ()
        head = (xT,)

    with tile.TileContext(nc) as tc, ExitStack() as rctx:
        pspool = rctx.enter_context(
            tc.tile_pool(name="ps", bufs=2, space=bass.MemorySpace.PSUM)
        )
        aps = head + (xTq, wqT, wkT, wvT, masks, ident, out) + extra + (pspool,)
        _emit_body(nc, tc, rctx, aps, version=version)

    nc.compile()
    return nc


def _pack(wT, gw=512):
    # [1024, cols] -> [128, cols//gw, 8, gw], the SBUF tile layout:
    # pk[p, g, d, o] = wT[d*128 + p, g*gw + o]; contiguous per partition
    cols = wT.shape[1]
    w4 = wT.reshape(NDC, P, cols // gw, gw)
    return np.ascontiguousarray(w4.transpose(1, 2, 0, 3))


def _prep_inputs(x, Wk, Wq, Wv, version=1):
    x = np.asarray(x, dtype=np.float32)
    wqT = _pack((np.asarray(Wq, np.float32).T / 32.0).astype(NPBF16))
    wkT = _pack(np.asarray(Wk, np.float32).T.astype(NPBF16))
    wvT = _pack(np.asarray(Wv, np.float32).T.astype(NPBF16))
    ident = np.eye(P, dtype=NPBF16)

    mask_by_h = {}
    maskT_by_h = {}
    for h in (0, 1):
        mk = np.empty((P, NQT, 256), np.float32)
        for j, t in enumerate(TILES[h]):
            base = LJS[j] - 256
            col = base + np.arange(256)[None, :]
            row = t * P + np.arange(P)[:, None]
            mk[:, j, :] = np.where(col <= row, 0.0, -1e30)
        mask_by_h[h] = mk
        # transposed masks for v4/v5: maskT[kp, p, parity, qc] for
        # k-block kb = 2p + parity vs q-tile t = TILES[h][p]:
        # allowed iff kb*128 + kp <= t*128 + qc
        mt = np.empty((P, NQT, 2, P), np.float32)
        kp = np.arange(P)[:, None]
        qc = np.arange(P)[None, :]
        for p, t in enumerate(TILES[h]):
            for parity in (0, 1):
                kb = 2 * p + parity
                mt[:, p, parity, :] = np.where(
                    kb * P + kp <= t * P + qc, 0.0, -1e30
                )
        maskT_by_h[h] = mt

    in_maps = []
    for c in range(8):
        b, h = c // 2, c % 2
        xTb = np.ascontiguousarray(x[b].T.astype(NPBF16))
        qcols = np.concatenate([np.arange(t * P, (t + 1) * P) for t in TILES[h]])
        xt_in = (
            xTb
            if version in (1, 3, 4)
            else xTb[:, h * (S // 2) : (h + 1) * (S // 2)]
        )
        m = {
            "xT": _pack(xt_in),
            "xTq": _pack(xTb[:, qcols]),
            "wqT": wqT,
            "wkT": wkT,
            "wvT": wvT,
        }
        if version in (4, 5):
            m["masksT"] = maskT_by_h[h]
            m["ones"] = np.ones((P, 1), np.float32)
        else:
            m["masks"] = mask_by_h[h]
            m["ident"] = ident
        if version in (3, 4):
            m["xTk"] = _pack(xTb[:, h * (S // 2) : (h + 1) * (S // 2)])
        in_maps.append(m)
    return in_maps


VERSION = int(os.environ.get("BASS_KERNEL_VERSION", "3"))


def kernel(x, Wk, Wq, Wv):
    global LAST_RESULTS
    if VERSION not in _COMPILED:
        _COMPILED[VERSION] = _build(VERSION)
    nc = _COMPILED[VERSION]
    in_maps = _prep_inputs(x, Wk, Wq, Wv, version=VERSION)
    trace = bool(int(os.environ.get("BASS_KERNEL_TRACE", "0")))
    res = run_bass_kernel_spmd(nc, in_maps, list(range(8)), trace=trace)
    LAST_RESULTS = res
    out = np.empty((B, S, D), np.float32)
    for c in range(8):
        b, h = c // 2, c % 2
        oc = res.results[c]["out"]
        for j, t in enumerate(TILES[h]):
            out[b, t * P : (t + 1) * P, :] = oc[j * P : (j + 1) * P, :]
    return out



# revision 20
# speedup vs baseline: 1.1465x; 1.1258x over previous
"""Causal attention (B=4, S=2048, D=1024) on 8 trn2 NeuronCores.

Sharding: core c = (batch b = c//2, query-group h = c%2). Default scheme
(version 3): each core K-projects its OWN key half (pairwise AllGather of KT
hides behind the V projection), V-projects its whole batch locally, and
Q-projects its own 8 query tiles of 128 rows. Tiles are interleaved (t % 4
in {0,3} for h=0, {1,2} for h=1) so both cores of a pair have the same
causal work profile and the SPMD program is identical on every core.

All matmul operands are bf16 (fp32 PSUM accumulation): halves DMA bytes and
SBUF footprint vs f32r, so x / K / V / Q all stay SBUF-resident. Inputs are
host-prepacked into the exact SBUF tile layout [128, G, 8, 512] so every
load is one DMA with 128 contiguous per-partition runs (DIRECT2D descriptor
generation is serialized per sequencer and costs ~5ns/descriptor — layout,
batching, and spreading issuance across the sync/scalar/gpsimd queues keep
it off the critical path). Collective-dependent readbacks ride the gpsimd
queue: a sync-queue wait on an unfinished collective deadlocks.

Device kernel per core:
  KT[o,k] = sum_d WkT[d,o] xTk[d,k]         k = own 1024 keys, then
                                            pairwise AllGather -> all 2048
  V[s,o]  = sum_d xT[d,s]  WvT[d,o]         s = 0..2047 (local, duplicated)
  QT[o,q] = sum_d WqT[d,o] xTq[d,q]         q = core's 1024 rows
                                            (Wq pre-scaled by 1/32 on host)
  per sorted q-tile position j (L = (2j+2)*128 keys, both h fit under L):
    S[q,k] = sum_o QT[o,q] KT[o,k];  last 256 cols += mask (covers diag
             block + the 128-col overhang the other h-core doesn't need)
    P = exp(S)  (no rowmax subtraction: |S| <= ~6, exp is fp32-safe;
             masked cols are -1e30 -> exp underflows to exactly 0)
    rowsum fused via activation accum_out
    C[q,:] = sum_k P^T[k,q] V[k,:]  (P^T via PE transpose, bf16)
    out = C * (1/rowsum)
"""

import os
import sys
from contextlib import ExitStack

import ml_dtypes
import numpy as np

sys.path.insert(0, "/opt/trn_rl_repo")

import concourse.bass as bass
import concourse.tile as tile
from concourse import bacc, mybir
from concourse.bass_utils import run_bass_kernel_spmd

F32 = mybir.dt.float32
BF16 = mybir.dt.bfloat16
NPBF16 = ml_dtypes.bfloat16
P = 128
B, S, D = 4, 2048, 1024
NDC = D // P                     # 8 contraction chunks of 128
NQT = 8                          # q-tiles of 128 rows per core
QCORE = NQT * P                  # 1024 q rows per core
TILES = {
    0: [t for t in range(16) if t % 4 in (0, 3)],
    1: [t for t in range(16) if t % 4 in (1, 2)],
}
# position j covers L_j = (2j+2)*128 key columns: the max over the two
# h-cores' causal needs at that sorted position; the mask input zeroes the
# per-core overhang (at most 128 cols, always inside the last 256).
LJS = [(2 * j + 2) * P for j in range(NQT)]

_COMPILED = {}
LAST_RESULTS = None


def _score_chunks(L):
    """Split L key cols into matmul chunks <=512; last chunk is the 256-wide
    mask window."""
    pre = L - 256
    chunks = []
    off = 0
    while pre - off >= 512:
        chunks.append((off, 512, False))
        off += 512
    if pre - off:
        chunks.append((off, pre - off, False))
    chunks.append((pre, 256, True))
    return chunks


def _emit_body(nc, tc, rctx, aps, version=1):
    if version == 1:
        xT, xTq, wqT, wkT, wvT, masks, ident, out, pspool = aps
        cc = None
    elif version == 2:
        (xT, xTq, wqT, wkT, wvT, masks, ident, out,
         ktag_in, ktag_out, vag_in, vag_out, pspool) = aps
        cc = [[0, 1], [2, 3], [4, 5], [6, 7]]
    else:  # version 3: K gathered pairwise, V+Q local
        (xT, xTk, xTq, wqT, wkT, wvT, masks, ident, out,
         ktag_in, ktag_out, pspool) = aps
        cc = [[0, 1], [2, 3], [4, 5], [6, 7]]
    KH = S if version in (1, 3) else S // 2  # value rows projected locally
    KK = S // 2 if version in (2, 3) else S  # key rows projected locally
    copy_ctr = [0]

    def copy_out(dst, src):
        # alternate PSUM->SBUF copies between vector and scalar engines
        copy_ctr[0] += 1
        if copy_ctr[0] % 2:
            nc.vector.tensor_copy(dst, src)
        else:
            nc.scalar.copy(dst, src)

    cpool = rctx.enter_context(tc.tile_pool(name="const", bufs=1))
    identsb = cpool.tile([P, P], BF16)
    masksb = cpool.tile([P, NQT, 256], F32)
    ktpool = rctx.enter_context(tc.tile_pool(name="ktp", bufs=1))
    kt_sb = ktpool.tile([P, NDC, S], BF16)     # KT: [o%128, o//128, k]
    vpool = rctx.enter_context(tc.tile_pool(name="vp", bufs=1))
    v_sb = vpool.tile([P, S // P, D], BF16)    # V: [s%128, s//128, o]
    qtpool = rctx.enter_context(tc.tile_pool(name="qtp", bufs=1))
    qt_sb = qtpool.tile([P, NDC, QCORE], BF16)  # QT: [o%128, o//128, q]

    with tc.tile_pool(name="wts", bufs=1) as wpool:
        wv_sb = wpool.tile([P, 2, NDC, 512], BF16)
        wq_sb = wpool.tile([P, 2, NDC, 512], BF16)
        xt_sb = wpool.tile([P, KH // 512, NDC, 512], BF16)  # [p, s//512, d, s%512]

        # inputs are host-prepacked as [128, G, 8, 512] (exact SBUF tile
        # layout, contiguous per partition): each 512-col group is one DMA
        # with 128 contiguous runs -> cheap descriptor generation
        def ld(eng, dst, w, g):
            eng.dma_start(dst[:, g], w[:, g])

        # ---- K phase: wk/xtk/kstg live only here (SBUF headroom) ---------
        with tc.tile_pool(name="kph", bufs=1) as kpool:
            wk_sb = kpool.tile([P, 2, NDC, 512], BF16)
            if version == 3:
                xtk_sb = kpool.tile([P, KK // 512, NDC, 512], BF16)  # own-half xT
            else:
                xtk_sb = xt_sb

            # issue order = first-use order; spread across the sync /
            # scalar / vector HWDGE queues so descriptor generation
            # doesn't serialize behind one sequencer
            xk_src = xTk if version == 3 else xT
            ld(nc.sync, wk_sb, wkT, 0)
            ld(nc.sync, xtk_sb, xk_src, 0)
            ld(nc.sync, wk_sb, wkT, 1)
            for g in range(1, KK // 512):
                ld(nc.sync, xtk_sb, xk_src, g)
            if version == 3:
                for g in range(KH // 512):
                    ld(nc.scalar, xt_sb, xT, g)
            ld(nc.sync, wv_sb, wvT, 0)
            ld(nc.sync, wv_sb, wvT, 1)
            ld(nc.scalar, wq_sb, wqT, 0)
            ld(nc.scalar, wq_sb, wqT, 1)
            nc.scalar.dma_start(identsb[:], ident[:])
            nc.scalar.dma_start(masksb[:], masks[:])

            # ---- K projection: KT[o, own keys] ---------------------------
            for ks in range(KK // 512):
                if version in (2, 3):
                    kst = kpool.tile([P, NDC, 512], BF16, tag="kstg", bufs=2)
                for c in range(NDC):
                    ps = pspool.tile([P, 512], F32, tag="mm", bufs=3)
                    for d in range(NDC):
                        nc.tensor.matmul(
                            ps[:],
                            wk_sb[:, c // 4, d, (c % 4) * P : (c % 4 + 1) * P],
                            xtk_sb[:, ks, d, :],
                            start=(d == 0),
                            stop=(d == NDC - 1),
                        )
                    if version == 1:
                        copy_out(kt_sb[:, c, ks * 512 : (ks + 1) * 512], ps[:])
                    else:
                        copy_out(kst[:, c, :], ps[:])
                if version in (2, 3):
                    # stage this k-chunk to DRAM for the pairwise gather
                    nc.gpsimd.dma_start(
                        ktag_in[:, ks * 512 : (ks + 1) * 512].rearrange(
                            "(c p) k -> p c k", p=P
                        ),
                        kst[:],
                    )
            if version in (2, 3):
                nc.gpsimd.collective_compute(
                    "AllGather", mybir.AluOpType.bypass, replica_groups=cc,
                    ins=[ktag_in[:]], outs=[ktag_out[:]],
                )
            if version == 3:
                # kt readback on the idle gpsimd queue, right behind the
                # collective: no sync-queue sem wait, so V/Q-proj DMA waits
                # can't get serialized behind the gather
                for r in range(2):
                    nc.gpsimd.dma_start(
                        kt_sb[:, :, r * KK : (r + 1) * KK],
                        ktag_out[r * D : (r + 1) * D, :].rearrange(
                            "(c p) k -> p c k", p=P
                        ),
                    )

        # ---- post-K pool: xtq (+ v2 staging) in the freed space ----------
        with tc.tile_pool(name="qph", bufs=1) as qpool:
            xtq_sb = qpool.tile([P, 2, NDC, 512], BF16)
            if version == 2:
                vstg = qpool.tile([P, KH // P, D], BF16)  # staged V
            ld(nc.scalar, xtq_sb, xTq, 0)
            ld(nc.scalar, xtq_sb, xTq, 1)

            # ---- V projection: V[own rows, o] --------------------------------
            vdst = vstg if version == 2 else v_sb
            for st_i in range(KH // P):
                for oh in range(2):
                    ps = pspool.tile([P, 512], F32, tag="mm", bufs=3)
                    for d in range(NDC):
                        nc.tensor.matmul(
                            ps[:],
                            xt_sb[:, st_i // 4, d, (st_i % 4) * P : (st_i % 4 + 1) * P],
                            wv_sb[:, oh, d, :],
                            start=(d == 0),
                            stop=(d == NDC - 1),
                        )
                    copy_out(vdst[:, st_i, oh * 512 : (oh + 1) * 512], ps[:])
                if version == 2 and st_i == 3:
                    # first V half staged -> gather it while the second half
                    # computes; kt readback rides the gpsimd queue in between
                    # (K gather already done, so it doesn't block the V gather).
                    # Gather A rows = [s 0:512 | s 1024:1536] (rank-major).
                    nc.sync.dma_start(
                        vag_in[0:512, :].rearrange("(t p) o -> p t o", p=P),
                        vstg[:, 0:4, :],
                    )
                    nc.gpsimd.collective_compute(
                        "AllGather", mybir.AluOpType.bypass, replica_groups=cc,
                        ins=[vag_in[0:512, :]], outs=[vag_out[0:1024, :]],
                    )
                    # kt readback on gpsimd behind gather A (K gather long
                    # done); keeping collective-dependent waits off the sync
                    # queue — a sync wait on an unfinished collective deadlocks
                    for r in range(2):
                        nc.gpsimd.dma_start(
                            kt_sb[:, :, r * KH : (r + 1) * KH],
                            ktag_out[r * D : (r + 1) * D, :].rearrange(
                                "(c p) k -> p c k", p=P
                            ),
                        )
                    # vagA available: v s-tiles 0-3 and 8-11
                    nc.gpsimd.dma_start(
                        v_sb[:, 0:4, :],
                        vag_out[0:512, :].rearrange("(t p) o -> p t o", p=P),
                    )
                    nc.gpsimd.dma_start(
                        v_sb[:, 8:12, :],
                        vag_out[512:1024, :].rearrange("(t p) o -> p t o", p=P),
                    )
            if version == 2:
                # Gather B rows = [s 512:1024 | s 1536:2048]
                nc.sync.dma_start(
                    vag_in[512:1024, :].rearrange("(t p) o -> p t o", p=P),
                    vstg[:, 4:8, :],
                )
                nc.gpsimd.collective_compute(
                    "AllGather", mybir.AluOpType.bypass, replica_groups=cc,
                    ins=[vag_in[512:1024, :]], outs=[vag_out[1024:2048, :]],
                )

            # ---- Q projection: QT[o, q] --------------------------------------
            for qs in range(QCORE // 512):
                for c in range(NDC):
                    ps = pspool.tile([P, 512], F32, tag="mm", bufs=3)
                    for d in range(NDC):
                        nc.tensor.matmul(
                            ps[:],
                            wq_sb[:, c // 4, d, (c % 4) * P : (c % 4 + 1) * P],
                            xtq_sb[:, qs, d, :],
                            start=(d == 0),
                            stop=(d == NDC - 1),
                        )
                    copy_out(qt_sb[:, c, qs * 512 : (qs + 1) * 512], ps[:])

            if version == 2:
                # vagB readback: v s-tiles 4-7 first (needed by PV from j=2 on)
                nc.gpsimd.dma_start(
                    v_sb[:, 4:8, :],
                    vag_out[1024:1536, :].rearrange("(t p) o -> p t o", p=P),
                )
                nc.gpsimd.dma_start(
                    v_sb[:, 12:16, :],
                    vag_out[1536:2048, :].rearrange("(t p) o -> p t o", p=P),
                )

    # ---- attention, software-pipelined per q-tile position ---------------
    with tc.tile_pool(name="sp", bufs=2) as sp, tc.tile_pool(
        name="pp", bufs=2
    ) as pp, tc.tile_pool(name="stats", bufs=4) as stp, tc.tile_pool(
        name="atp", bufs=4
    ) as atp, tc.tile_pool(name="cp", bufs=2) as cp:
        state = {}

        def emit_scores(j):
            L = LJS[j]
            ssb = sp.tile([P, L], F32, tag="ssb", name=f"ssb{j}")
            for off, w, is_mask in _score_chunks(L):
                ps = pspool.tile([P, 512], F32, tag="mm", bufs=3)
                for c in range(NDC):
                    nc.tensor.matmul(
                        ps[:, :w],
                        qt_sb[:, c, j * P : (j + 1) * P],
                        kt_sb[:, c, off : off + w],
                        start=(c == 0),
                        stop=(c == NDC - 1),
                    )
                if is_mask:
                    nc.vector.tensor_add(
                        ssb[:, off : off + w], ps[:, :w], masksb[:, j, :]
                    )
                else:
                    copy_out(ssb[:, off : off + w], ps[:, :w])
            state[j] = ssb

        def emit_softmax_pv(j):
            L = LJS[j]
            ssb = state.pop(j)
            psb = pp.tile([P, L], BF16, tag="psb", name=f"psb{j}")
            sumv = stp.tile([P, 1], F32, tag="sumv", name=f"sumv{j}")
            nc.scalar.activation(
                psb[:],
                ssb[:],
                mybir.ActivationFunctionType.Exp,
                scale=1.0,
                accum_out=sumv[:],
            )
            rcp = stp.tile([P, 1], F32, tag="rcp", name=f"rcp{j}")
            nc.vector.reciprocal(rcp[:], sumv[:])

            co0 = pspool.tile([P, 512], F32, tag="co", bufs=2, name=f"co0_{j}")
            co1 = pspool.tile([P, 512], F32, tag="co", bufs=2, name=f"co1_{j}")
            nkt = L // P
            for k in range(nkt):
                tp = pspool.tile([P, P], BF16, tag="tp", bufs=3, name=f"tp{j}_{k}")
                nc.tensor.transpose(tp[:], psb[:, k * P : (k + 1) * P], identsb[:])
                at = atp.tile([P, P], BF16, tag="at", name=f"at{j}_{k}")
                copy_out(at[:], tp[:])
                nc.tensor.matmul(
                    co0[:], at[:], v_sb[:, k, 0:512],
                    start=(k == 0), stop=(k == nkt - 1),
                )
                nc.tensor.matmul(
                    co1[:], at[:], v_sb[:, k, 512:1024],
                    start=(k == 0), stop=(k == nkt - 1),
                )
            csb = cp.tile([P, D], F32, tag="csb", name=f"csb{j}")
            nc.vector.tensor_scalar_mul(csb[:, 0:512], co0[:], rcp[:])
            nc.sync.dma_start(out[j * P : (j + 1) * P, 0:512], csb[:, 0:512])
            nc.scalar.activation(
                csb[:, 512:1024],
                co1[:],
                mybir.ActivationFunctionType.Copy,
                scale=rcp[:],
            )
            nc.sync.dma_start(out[j * P : (j + 1) * P, 512:1024], csb[:, 512:1024])

        emit_scores(0)
        for j in range(1, NQT):
            emit_scores(j)
            emit_softmax_pv(j - 1)
        emit_softmax_pv(NQT - 1)


def _emit_body_v45(nc, tc, rctx, aps, version=4):
    """S^T-direct attention (no PE transposes). version 4: V projected
    locally for all 2048 rows (duplicated per pair, like v3). version 5:
    V projection sharded by rows + pairwise AllGather (like v2).

    Attention works on two quads of 4 sorted q-tile positions each. For
    quad g (positions p = 4g..4g+3), scores are computed TRANSPOSED:
      S^T[k, q] = sum_o KT[o, k-block] QT[o, q]   (k on partitions)
    per 128-row k-block kb, over the q columns of all members that
    causally need kb (member p needs kb <= 2p+1 across both h cores; the
    h-specific overhang + diagonal triangle are handled by a per-core
    maskT input added on DVE). exp(S^T) lands directly in the PV
    stationary layout, so the 72 PE transposes + PSUM->SBUF P^T copies
    of v3 disappear. The softmax row-sum rides a 1-column ones-matmul
    that reuses the already-loaded P^T stationary.
    """
    if version == 4:
        (xT, xTk, xTq, wqT, wkT, wvT, masksT, ones, out,
         ktag_in, ktag_out, pspool) = aps
    else:
        (xT, xTq, wqT, wkT, wvT, masksT, ones, out,
         ktag_in, ktag_out, vag_in, vag_out, pspool) = aps
    cc = [[0, 1], [2, 3], [4, 5], [6, 7]]
    KH = S if version == 4 else S // 2   # V rows projected locally
    KK = S // 2                          # K rows projected locally
    copy_ctr = [0]

    def copy_out(dst, src):
        copy_ctr[0] += 1
        if copy_ctr[0] % 2:
            nc.vector.tensor_copy(dst, src)
        else:
            nc.scalar.copy(dst, src)

    ktpool = rctx.enter_context(tc.tile_pool(name="ktp", bufs=1))
    # KT chunk-major: [o%128, key-chunk(512), o//128, k%512] so each
    # gather-readback DMA writes one contiguous chunk (no interleaved
    # address ranges -> no false deps stalling the first scores)
    kt_sb = ktpool.tile([P, 4, NDC, 512], BF16)
    vpool = rctx.enter_context(tc.tile_pool(name="vp", bufs=1))
    v_sb = vpool.tile([P, S // P, D], BF16)    # V: [s%128, s//128, o]
    qtpool = rctx.enter_context(tc.tile_pool(name="qtp", bufs=1))
    qt_sb = qtpool.tile([P, NDC, QCORE], BF16)  # QT: [o%128, o//128, q]

    with tc.tile_pool(name="wts", bufs=1) as wpool:
        wv_sb = wpool.tile([P, 2, NDC, 512], BF16)
        wq_sb = wpool.tile([P, 2, NDC, 512], BF16)
        xtq_sb = wpool.tile([P, 2, NDC, 512], BF16)
        xt_sb = wpool.tile([P, KH // 512, NDC, 512], BF16)

        def ld(eng, dst, w, g):
            eng.dma_start(dst[:, g], w[:, g])

        def ld2(eng_a, eng_b, dst, w, g):
            # split one 512-col group across two DMA queues (halves the
            # startup-critical load latency); only sync/scalar/gpsimd
            # have DMA queues
            eng_a.dma_start(dst[:, g, 0:4], w[:, g, 0:4])
            eng_b.dma_start(dst[:, g, 4:8], w[:, g, 4:8])

        # ---- K phase ----------------------------------------------------
        with tc.tile_pool(name="kph", bufs=1) as kpool:
            wk_sb = kpool.tile([P, 2, NDC, 512], BF16)
            if version == 4:
                xtk_sb = kpool.tile([P, KK // 512, NDC, 512], BF16)
                xk_src = xTk
            else:
                xtk_sb = xt_sb
                xk_src = xT

            # DMA issuance lives in the issuing ENGINE's instruction
            # FIFO with flow-control waits, so loads must NEVER ride the
            # scalar/vector engines (they do the PSUM copy-outs — a
            # queued load blocks them and stalls the PE). All input
            # loads go on sync + gpsimd only, in first-use order; the
            # startup-critical wk/x g0 are split across both queues.
            # gpsimd carries nothing after xtk g1 (reserved for K
            # staging + collective + readbacks).
            def ldp(eng, dst, w, g, d0, d1):
                eng.dma_start(dst[:, g, d0:d1], w[:, g, d0:d1])

            ldp(nc.sync, wk_sb, wkT, 0, 0, 4)
            ldp(nc.gpsimd, wk_sb, wkT, 0, 4, 8)
            ldp(nc.sync, xtk_sb, xk_src, 0, 0, 4)
            ldp(nc.gpsimd, xtk_sb, xk_src, 0, 4, 8)
            ld(nc.sync, wk_sb, wkT, 1)
            if version == 4:
                ld(nc.gpsimd, xtk_sb, xTk, 1)
                rest = [(wv_sb, wvT, 0), (wv_sb, wvT, 1),
                        (xt_sb, xT, 0), (xt_sb, xT, 1),
                        (xt_sb, xT, 2), (xt_sb, xT, 3)]
            else:
                ld(nc.gpsimd, xt_sb, xT, 1)
                rest = [(wv_sb, wvT, 0), (wv_sb, wvT, 1)]
            rest += [(xtq_sb, xTq, 0), (xtq_sb, xTq, 1),
                     (wq_sb, wqT, 0), (wq_sb, wqT, 1)]
            for dst, w, g in rest:
                ld(nc.sync, dst, w, g)

            # K projection: KT[o, own keys]; stage per 4-c half (512KB
            # DMAs: descriptor generation on the gpsimd engine costs
            # ~1us per DMA, so fewer+bigger beats 16x128KB) with a
            # 3-buffered half-size staging tile so the next group's
            # copy-outs never wait on a staging drain
            for ks in range(KK // 512):
                for ch in range(2):
                    kst = kpool.tile(
                        [P, 4, 512], BF16, tag="kstg", bufs=3,
                        name=f"kst{ks}_{ch}",
                    )
                    for c4 in range(4):
                        c = ch * 4 + c4
                        ps = pspool.tile([P, 512], F32, tag="mm", bufs=3)
                        for d in range(NDC):
                            nc.tensor.matmul(
                                ps[:],
                                wk_sb[:, c // 4, d, (c % 4) * P : (c % 4 + 1) * P],
                                xtk_sb[:, ks, d, :],
                                start=(d == 0),
                                stop=(d == NDC - 1),
                            )
                        copy_out(kst[:, c4, :], ps[:])
                    nc.gpsimd.dma_start(
                        ktag_in[
                            ch * 512 : (ch + 1) * 512,
                            ks * 512 : (ks + 1) * 512,
                        ].rearrange("(c p) k -> p c k", p=P),
                        kst[:],
                    )
            nc.gpsimd.collective_compute(
                "AllGather", mybir.AluOpType.bypass, replica_groups=cc,
                ins=[ktag_in[:]], outs=[ktag_out[:]],
            )
            # readback in 4 chunked DMAs; chunk ci = global keys
            # ci*512:(ci+1)*512 = ktag_out[rank(ci)*D rows, (ci%2) cols]
            for ci in range(4):
                r, half = ci // 2, ci % 2
                nc.gpsimd.dma_start(
                    kt_sb[:, ci],
                    ktag_out[
                        r * D : (r + 1) * D, half * 512 : (half + 1) * 512
                    ].rearrange("(c p) k -> p c k", p=P),
                )

        # ---- V + Q projections ------------------------------------------
        with tc.tile_pool(name="qph", bufs=1) as qpool:
            if version == 5:
                vstg = qpool.tile([P, KH // P, D], BF16)

            vdst = vstg if version == 5 else v_sb
            for st_i in range(KH // P):
                for oh in range(2):
                    ps = pspool.tile([P, 512], F32, tag="mm", bufs=3)
                    for d in range(NDC):
                        nc.tensor.matmul(
                            ps[:],
                            xt_sb[:, st_i // 4, d, (st_i % 4) * P : (st_i % 4 + 1) * P],
                            wv_sb[:, oh, d, :],
                            start=(d == 0),
                            stop=(d == NDC - 1),
                        )
                    copy_out(vdst[:, st_i, oh * 512 : (oh + 1) * 512], ps[:])
                if version == 5 and st_i == 3:
                    nc.sync.dma_start(
                        vag_in[0:512, :].rearrange("(t p) o -> p t o", p=P),
                        vstg[:, 0:4, :],
                    )
                    nc.gpsimd.collective_compute(
                        "AllGather", mybir.AluOpType.bypass, replica_groups=cc,
                        ins=[vag_in[0:512, :]], outs=[vag_out[0:1024, :]],
                    )
                    # gather A yields global s-tiles 0-3 (rank0) + 8-11
                    nc.gpsimd.dma_start(
                        v_sb[:, 0:4, :],
                        vag_out[0:512, :].rearrange("(t p) o -> p t o", p=P),
                    )
                    nc.gpsimd.dma_start(
                        v_sb[:, 8:12, :],
                        vag_out[512:1024, :].rearrange("(t p) o -> p t o", p=P),
                    )
            if version == 5:
                nc.sync.dma_start(
                    vag_in[512:1024, :].rearrange("(t p) o -> p t o", p=P),
                    vstg[:, 4:8, :],
                )
                nc.gpsimd.collective_compute(
                    "AllGather", mybir.AluOpType.bypass, replica_groups=cc,
                    ins=[vag_in[512:1024, :]], outs=[vag_out[1024:2048, :]],
                )
                nc.gpsimd.dma_start(
                    v_sb[:, 4:8, :],
                    vag_out[1024:1536, :].rearrange("(t p) o -> p t o", p=P),
                )
                nc.gpsimd.dma_start(
                    v_sb[:, 12:16, :],
                    vag_out[1536:2048, :].rearrange("(t p) o -> p t o", p=P),
                )

            # Q projection: QT[o, q]
            for qs in range(QCORE // 512):
                for c in range(NDC):
                    ps = pspool.tile([P, 512], F32, tag="mm", bufs=3)
                    for d in range(NDC):
                        nc.tensor.matmul(
                            ps[:],
                            wq_sb[:, c // 4, d, (c % 4) * P : (c % 4 + 1) * P],
                            xtq_sb[:, qs, d, :],
                            start=(d == 0),
                            stop=(d == NDC - 1),
                        )
                    copy_out(qt_sb[:, c, qs * 512 : (qs + 1) * 512], ps[:])

    # ---- attention: S^T-direct over two quads ----------------------------
    with tc.tile_pool(name="ptq", bufs=1) as ptpool, tc.tile_pool(
        name="stats", bufs=4
    ) as stp, tc.tile_pool(name="cp", bufs=2) as cp, tc.tile_pool(
        name="accp", bufs=1
    ) as accp:
        onesb = accp.tile([P, 1], F32)
        maskTsb = accp.tile([P, NQT, 2, P], F32)
        nc.sync.dma_start(onesb[:], ones[:])
        nc.sync.dma_start(maskTsb[:], masksT[:])
        pt = {
            g: ptpool.tile(
                [P, 8 * g + 8, 512], BF16, tag=f"ptq{g}", name=f"ptq{g}"
            )
            for g in (0, 1)
        }
        # per-quad f32 rowsum accumulators [k-part, member q cols]; the
        # partial-width adds ([qoff:512]) accumulate exactly each
        # member's causal kb range
        acc = {
            g: accp.tile([P, 512], F32, tag=f"acc{g}", name=f"acc{g}")
            for g in (0, 1)
        }

        def emit_scores_st(g, kbs):
            for kb in kbs:
                p_lo = max(4 * g, kb // 2)
                qoff = (p_lo - 4 * g) * P
                st = pspool.tile([P, 512], F32, tag="mm", bufs=3)
                for c in range(NDC):
                    nc.tensor.matmul(
                        st[:, qoff:512],
                        kt_sb[:, kb // 4, c, (kb % 4) * P : (kb % 4 + 1) * P],
                        qt_sb[:, c, g * 512 + qoff : (g + 1) * 512],
                        start=(c == 0),
                        stop=(c == NDC - 1),
                    )
                pm = kb // 2  # member whose diagonal/overhang region this is
                if pm >= 4 * g:
                    mo = (pm - 4 * g) * P
                    nc.vector.tensor_add(
                        st[:, mo : mo + P], st[:, mo : mo + P],
                        maskTsb[:, pm, kb % 2, :],
                    )
                nc.scalar.activation(
                    pt[g][:, kb, qoff:512],
                    st[:, qoff:512],
                    mybir.ActivationFunctionType.Exp,
                    scale=1.0,
                )
                if kb == 0:
                    nc.vector.tensor_copy(acc[g][:, 0:512], pt[g][:, 0, 0:512])
                else:
                    nc.vector.tensor_add(
                        acc[g][:, qoff:512], acc[g][:, qoff:512],
                        pt[g][:, kb, qoff:512],
                    )

        def emit_pv(g, members):
            for p in members:
                qo = (p - 4 * g) * P
                co0 = pspool.tile([P, 512], F32, tag="co", bufs=4, name=f"co0_{p}")
                co1 = pspool.tile([P, 512], F32, tag="co", bufs=4, name=f"co1_{p}")
                last = 2 * p + 1
                for kb in range(last + 1):
                    lhsT = pt[g][:, kb, qo : qo + P]
                    nc.tensor.matmul(co0[:], lhsT, v_sb[:, kb, 0:512],
                                     start=(kb == 0), stop=(kb == last))
                    nc.tensor.matmul(co1[:], lhsT, v_sb[:, kb, 512:1024],
                                     start=(kb == 0), stop=(kb == last))
                # cross-partition rowsum: ones-contraction of the f32
                # accumulator chunk (q lands on partitions)
                rs = pspool.tile([P, 1], F32, tag="rs", bufs=1, name=f"rs{p}")
                nc.tensor.matmul(rs[:], acc[g][:, qo : qo + P], onesb[:],
                                 start=True, stop=True)
                rcp = stp.tile([P, 1], F32, tag="rcp", name=f"rcp{p}")
                nc.vector.reciprocal(rcp[:], rs[:])
                csb = cp.tile([P, D], F32, tag="csb", name=f"csb{p}")
                for ch in range(4):
                    lo, hi = ch * 256, (ch + 1) * 256
                    src = co0 if ch < 2 else co1
                    slo, shi = (lo, hi) if ch < 2 else (lo - 512, hi - 512)
                    if ch % 2 == 0:
                        nc.vector.tensor_scalar_mul(
                            csb[:, lo:hi], src[:, slo:shi], rcp[:]
                        )
                    else:
                        nc.scalar.activation(
                            csb[:, lo:hi], src[:, slo:shi],
                            mybir.ActivationFunctionType.Copy, scale=rcp[:],
                        )
                    nc.sync.dma_start(
                        out[p * P : (p + 1) * P, lo:hi], csb[:, lo:hi]
                    )

        # order: quad0 scores, quad1 scores (first half), quad0 PV m0/m1,
        # quad1 scores (rest), quad0 PV m2/m3, quad1 PV — PE stays dense
        # while the K/V gathers get maximum slack
        emit_scores_st(0, range(8))
        emit_scores_st(1, range(8))
        emit_pv(0, (0, 1))
        emit_scores_st(1, range(8, 16))
        emit_pv(0, (2, 3))
        emit_pv(1, (4, 5, 6, 7))


def _build(version=1):
    nc = bacc.Bacc("TRN2", target_bir_lowering=False, debug=False, num_devices=8)

    kh = S if version in (1, 3, 4) else S // 2

    def packed(name, cols):
        # host-prepacked [128, cols//512, 8, 512] input (SBUF tile layout)
        return nc.dram_tensor(
            name, [P, cols // 512, NDC, 512], BF16, kind="ExternalInput"
        ).ap()

    xT = packed("xT", kh)
    xTq = packed("xTq", QCORE)
    wqT = packed("wqT", D)
    wkT = packed("wkT", D)
    wvT = packed("wvT", D)
    out = nc.dram_tensor("out", [QCORE, D], F32, kind="ExternalOutput").ap()
    if version in (4, 5):
        masksT = nc.dram_tensor(
            "masksT", [P, NQT, 2, P], F32, kind="ExternalInput"
        ).ap()
        ones = nc.dram_tensor("ones", [P, 1], F32, kind="ExternalInput").ap()
        ktag_in = nc.dram_tensor("ktag_in", [D, S // 2], BF16).ap()
        ktag_out = nc.dram_tensor("ktag_out", [2 * D, S // 2], BF16).ap()
        if version == 4:
            xTk = packed("xTk", S // 2)
            aps_head = (xT, xTk, xTq, wqT, wkT, wvT, masksT, ones, out,
                        ktag_in, ktag_out)
        else:
            vag_in = nc.dram_tensor("vag_in", [S // 2, D], BF16).ap()
            vag_out = nc.dram_tensor("vag_out", [S, D], BF16).ap()
            aps_head = (xT, xTq, wqT, wkT, wvT, masksT, ones, out,
                        ktag_in, ktag_out, vag_in, vag_out)
        with tile.TileContext(nc) as tc, ExitStack() as rctx:
            pspool = rctx.enter_context(
                tc.tile_pool(name="ps", bufs=2, space=bass.MemorySpace.PSUM)
            )
            _emit_body_v45(nc, tc, rctx, aps_head + (pspool,), version=version)
        nc.compile()
        return nc

    masks = nc.dram_tensor("masks", [P, NQT, 256], F32, kind="ExternalInput").ap()
    ident = nc.dram_tensor("ident", [P, P], BF16, kind="ExternalInput").ap()
    if version == 2:
        ktag_in = nc.dram_tensor("ktag_in", [D, S // 2], BF16).ap()
        ktag_out = nc.dram_tensor("ktag_out", [2 * D, S // 2], BF16).ap()
        vag_in = nc.dram_tensor("vag_in", [S // 2, D], BF16).ap()
        vag_out = nc.dram_tensor("vag_out", [S, D], BF16).ap()
        extra = (ktag_in, ktag_out, vag_in, vag_out)
        head = (xT,)
    elif version == 3:
        xTk = packed("xTk", S // 2)
        ktag_in = nc.dram_tensor("ktag_in", [D, S // 2], BF16).ap()
        ktag_out = nc.dram_tensor("ktag_out", [2 * D, S // 2], BF16).ap()
        extra = (ktag_in, ktag_out)
        head = (xT, xTk)
    else:
        extra = ()
        head = (xT,)

    with tile.TileContext(nc) as tc, ExitStack() as rctx:
        pspool = rctx.enter_context(
            tc.tile_pool(name="ps", bufs=2, space=bass.MemorySpace.PSUM)
        )
        aps = head + (xTq, wqT, wkT, wvT, masks, ident, out) + extra + (pspool,)
        _emit_body(nc, tc, rctx, aps, version=version)

    nc.compile()
    return nc


def _pack(wT, gw=512):
    # [1024, cols] -> [128, cols//gw, 8, gw], the SBUF tile layout:
    # pk[p, g, d, o] = wT[d*128 + p, g*gw + o]; contiguous per partition
    cols = wT.shape[1]
    w4 = wT.reshape(NDC, P, cols // gw, gw)
    return np.ascontiguousarray(w4.transpose(1, 2, 0, 3))


def _prep_inputs(x, Wk, Wq, Wv, version=1):
    x = np.asarray(x, dtype=np.float32)
    wqT = _pack((np.asarray(Wq, np.float32).T / 32.0).astype(NPBF16))
    wkT = _pack(np.asarray(Wk, np.float32).T.astype(NPBF16))
    wvT = _pack(np.asarray(Wv, np.float32).T.astype(NPBF16))
    ident = np.eye(P, dtype=NPBF16)

    mask_by_h = {}
    maskT_by_h = {}
    for h in (0, 1):
        mk = np.empty((P, NQT, 256), np.float32)
        for j, t in enumerate(TILES[h]):
            base = LJS[j] - 256
            col = base + np.arange(256)[None, :]
            row = t * P + np.arange(P)[:, None]
            mk[:, j, :] = np.where(col <= row, 0.0, -1e30)
        mask_by_h[h] = mk
        # transposed masks for v4/v5: maskT[kp, p, parity, qc] for
        # k-block kb = 2p + parity vs q-tile t = TILES[h][p]:
        # allowed iff kb*128 + kp <= t*128 + qc
        mt = np.empty((P, NQT, 2, P), np.float32)
        kp = np.arange(P)[:, None]
        qc = np.arange(P)[None, :]
        for p, t in enumerate(TILES[h]):
            for parity in (0, 1):
                kb = 2 * p + parity
                mt[:, p, parity, :] = np.where(
                    kb * P + kp <= t * P + qc, 0.0, -1e30
                )
        maskT_by_h[h] = mt

    in_maps = []
    for c in range(8):
        b, h = c // 2, c % 2
        xTb = np.ascontiguousarray(x[b].T.astype(NPBF16))
        qcols = np.concatenate([np.arange(t * P, (t + 1) * P) for t in TILES[h]])
        xt_in = (
            xTb
            if version in (1, 3, 4)
            else xTb[:, h * (S // 2) : (h + 1) * (S // 2)]
        )
        m = {
            "xT": _pack(xt_in),
            "xTq": _pack(xTb[:, qcols]),
            "wqT": wqT,
            "wkT": wkT,
            "wvT": wvT,
        }
        if version in (4, 5):
            m["masksT"] = maskT_by_h[h]
            m["ones"] = np.ones((P, 1), np.float32)
        else:
            m["masks"] = mask_by_h[h]
            m["ident"] = ident
        if version in (3, 4):
            m["xTk"] = _pack(xTb[:, h * (S // 2) : (h + 1) * (S // 2)])
        in_maps.append(m)
    return in_maps


VERSION = int(os.environ.get("BASS_KERNEL_VERSION", "3"))


def kernel(x, Wk, Wq, Wv):
    global LAST_RESULTS
    if VERSION not in _COMPILED:
        _COMPILED[VERSION] = _build(VERSION)
    nc = _COMPILED[VERSION]
    in_maps = _prep_inputs(x, Wk, Wq, Wv, version=VERSION)
    trace = bool(int(os.environ.get("BASS_KERNEL_TRACE", "0")))
    res = run_bass_kernel_spmd(nc, in_maps, list(range(8)), trace=trace)
    LAST_RESULTS = res
    out = np.empty((B, S, D), np.float32)
    for c in range(8):
        b, h = c // 2, c % 2
        oc = res.results[c]["out"]
        for j, t in enumerate(TILES[h]):
            out[b, t * P : (t + 1) * P, :] = oc[j * P : (j + 1) * P, :]
    return out



# revision 33
# speedup vs baseline: 1.2688x; 1.1066x over previous
"""Causal attention (B=4, S=2048, D=1024) on 8 trn2 NeuronCores.

Sharding: core c = (batch b = c//2, query-group h = c%2). Default scheme
(version 3): each core K-projects its OWN key half (pairwise AllGather of KT
hides behind the V projection), V-projects its whole batch locally, and
Q-projects its own 8 query tiles of 128 rows. Tiles are interleaved (t % 4
in {0,3} for h=0, {1,2} for h=1) so both cores of a pair have the same
causal work profile and the SPMD program is identical on every core.

All matmul operands are bf16 (fp32 PSUM accumulation): halves DMA bytes and
SBUF footprint vs f32r, so x / K / V / Q all stay SBUF-resident. Inputs are
host-prepacked into the exact SBUF tile layout [128, G, 8, 512] so every
load is one DMA with 128 contiguous per-partition runs (DIRECT2D descriptor
generation is serialized per sequencer and costs ~5ns/descriptor — layout,
batching, and spreading issuance across the sync/scalar/gpsimd queues keep
it off the critical path). Collective-dependent readbacks ride the gpsimd
queue: a sync-queue wait on an unfinished collective deadlocks.

Device kernel per core:
  KT[o,k] = sum_d WkT[d,o] xTk[d,k]         k = own 1024 keys, then
                                            pairwise AllGather -> all 2048
  V[s,o]  = sum_d xT[d,s]  WvT[d,o]         s = 0..2047 (local, duplicated)
  QT[o,q] = sum_d WqT[d,o] xTq[d,q]         q = core's 1024 rows
                                            (Wq pre-scaled by 1/32 on host)
  per sorted q-tile position j (L = (2j+2)*128 keys, both h fit under L):
    S[q,k] = sum_o QT[o,q] KT[o,k];  last 256 cols += mask (covers diag
             block + the 128-col overhang the other h-core doesn't need)
    P = exp(S)  (no rowmax subtraction: |S| <= ~6, exp is fp32-safe;
             masked cols are -1e30 -> exp underflows to exactly 0)
    rowsum fused via activation accum_out
    C[q,:] = sum_k P^T[k,q] V[k,:]  (P^T via PE transpose, bf16)
    out = C * (1/rowsum)
"""

import os
import sys
from contextlib import ExitStack

import ml_dtypes
import numpy as np

sys.path.insert(0, "/opt/trn_rl_repo")

import concourse.bass as bass
import concourse.tile as tile
from concourse import bacc, mybir
from concourse.bass_utils import run_bass_kernel_spmd

F32 = mybir.dt.float32
BF16 = mybir.dt.bfloat16
NPBF16 = ml_dtypes.bfloat16
P = 128
B, S, D = 4, 2048, 1024
NDC = D // P                     # 8 contraction chunks of 128
NQT = 8                          # q-tiles of 128 rows per core
QCORE = NQT * P                  # 1024 q rows per core
TILES = {
    0: [t for t in range(16) if t % 4 in (0, 3)],
    1: [t for t in range(16) if t % 4 in (1, 2)],
}
# position j covers L_j = (2j+2)*128 key columns: the max over the two
# h-cores' causal needs at that sorted position; the mask input zeroes the
# per-core overhang (at most 128 cols, always inside the last 256).
LJS = [(2 * j + 2) * P for j in range(NQT)]

_COMPILED = {}
LAST_RESULTS = None


def _score_chunks(L):
    """Split L key cols into matmul chunks <=512; last chunk is the 256-wide
    mask window."""
    pre = L - 256
    chunks = []
    off = 0
    while pre - off >= 512:
        chunks.append((off, 512, False))
        off += 512
    if pre - off:
        chunks.append((off, pre - off, False))
    chunks.append((pre, 256, True))
    return chunks


def _emit_body(nc, tc, rctx, aps, version=1):
    if version == 1:
        xT, xTq, wqT, wkT, wvT, masks, ident, out, pspool = aps
        cc = None
    elif version == 2:
        (xT, xTq, wqT, wkT, wvT, masks, ident, out,
         ktag_in, ktag_out, vag_in, vag_out, pspool) = aps
        cc = [[0, 1], [2, 3], [4, 5], [6, 7]]
    else:  # version 3: K gathered pairwise, V+Q local
        (xT, xTk, xTq, wqT, wkT, wvT, masks, ident, out,
         ktag_in, ktag_out, pspool) = aps
        cc = [[0, 1], [2, 3], [4, 5], [6, 7]]
    KH = S if version in (1, 3) else S // 2  # value rows projected locally
    KK = S // 2 if version in (2, 3) else S  # key rows projected locally
    copy_ctr = [0]

    def copy_out(dst, src):
        # alternate PSUM->SBUF copies between vector and scalar engines
        copy_ctr[0] += 1
        if copy_ctr[0] % 2:
            nc.vector.tensor_copy(dst, src)
        else:
            nc.scalar.copy(dst, src)

    cpool = rctx.enter_context(tc.tile_pool(name="const", bufs=1))
    identsb = cpool.tile([P, P], BF16)
    masksb = cpool.tile([P, NQT, 256], F32)
    ktpool = rctx.enter_context(tc.tile_pool(name="ktp", bufs=1))
    kt_sb = ktpool.tile([P, NDC, S], BF16)     # KT: [o%128, o//128, k]
    vpool = rctx.enter_context(tc.tile_pool(name="vp", bufs=1))
    v_sb = vpool.tile([P, S // P, D], BF16)    # V: [s%128, s//128, o]
    qtpool = rctx.enter_context(tc.tile_pool(name="qtp", bufs=1))
    qt_sb = qtpool.tile([P, NDC, QCORE], BF16)  # QT: [o%128, o//128, q]

    with tc.tile_pool(name="wts", bufs=1) as wpool:
        wv_sb = wpool.tile([P, 2, NDC, 512], BF16)
        wq_sb = wpool.tile([P, 2, NDC, 512], BF16)
        xt_sb = wpool.tile([P, KH // 512, NDC, 512], BF16)  # [p, s//512, d, s%512]

        # inputs are host-prepacked as [128, G, 8, 512] (exact SBUF tile
        # layout, contiguous per partition): each 512-col group is one DMA
        # with 128 contiguous runs -> cheap descriptor generation
        def ld(eng, dst, w, g):
            eng.dma_start(dst[:, g], w[:, g])

        # ---- K phase: wk/xtk/kstg live only here (SBUF headroom) ---------
        with tc.tile_pool(name="kph", bufs=1) as kpool:
            wk_sb = kpool.tile([P, 2, NDC, 512], BF16)
            if version == 3:
                xtk_sb = kpool.tile([P, KK // 512, NDC, 512], BF16)  # own-half xT
            else:
                xtk_sb = xt_sb

            # issue order = first-use order; spread across the sync /
            # scalar / vector HWDGE queues so descriptor generation
            # doesn't serialize behind one sequencer
            xk_src = xTk if version == 3 else xT
            ld(nc.sync, wk_sb, wkT, 0)
            ld(nc.sync, xtk_sb, xk_src, 0)
            ld(nc.sync, wk_sb, wkT, 1)
            for g in range(1, KK // 512):
                ld(nc.sync, xtk_sb, xk_src, g)
            if version == 3:
                for g in range(KH // 512):
                    ld(nc.scalar, xt_sb, xT, g)
            ld(nc.sync, wv_sb, wvT, 0)
            ld(nc.sync, wv_sb, wvT, 1)
            ld(nc.scalar, wq_sb, wqT, 0)
            ld(nc.scalar, wq_sb, wqT, 1)
            nc.scalar.dma_start(identsb[:], ident[:])
            nc.scalar.dma_start(masksb[:], masks[:])

            # ---- K projection: KT[o, own keys] ---------------------------
            for ks in range(KK // 512):
                if version in (2, 3):
                    kst = kpool.tile([P, NDC, 512], BF16, tag="kstg", bufs=2)
                for c in range(NDC):
                    ps = pspool.tile([P, 512], F32, tag="mm", bufs=3)
                    for d in range(NDC):
                        nc.tensor.matmul(
                            ps[:],
                            wk_sb[:, c // 4, d, (c % 4) * P : (c % 4 + 1) * P],
                            xtk_sb[:, ks, d, :],
                            start=(d == 0),
                            stop=(d == NDC - 1),
                        )
                    if version == 1:
                        copy_out(kt_sb[:, c, ks * 512 : (ks + 1) * 512], ps[:])
                    else:
                        copy_out(kst[:, c, :], ps[:])
                if version in (2, 3):
                    # stage this k-chunk to DRAM for the pairwise gather
                    nc.gpsimd.dma_start(
                        ktag_in[:, ks * 512 : (ks + 1) * 512].rearrange(
                            "(c p) k -> p c k", p=P
                        ),
                        kst[:],
                    )
            if version in (2, 3):
                nc.gpsimd.collective_compute(
                    "AllGather", mybir.AluOpType.bypass, replica_groups=cc,
                    ins=[ktag_in[:]], outs=[ktag_out[:]],
                )
            if version == 3:
                # kt readback on the idle gpsimd queue, right behind the
                # collective: no sync-queue sem wait, so V/Q-proj DMA waits
                # can't get serialized behind the gather
                for r in range(2):
                    nc.gpsimd.dma_start(
                        kt_sb[:, :, r * KK : (r + 1) * KK],
                        ktag_out[r * D : (r + 1) * D, :].rearrange(
                            "(c p) k -> p c k", p=P
                        ),
                    )

        # ---- post-K pool: xtq (+ v2 staging) in the freed space ----------
        with tc.tile_pool(name="qph", bufs=1) as qpool:
            xtq_sb = qpool.tile([P, 2, NDC, 512], BF16)
            if version == 2:
                vstg = qpool.tile([P, KH // P, D], BF16)  # staged V
            ld(nc.scalar, xtq_sb, xTq, 0)
            ld(nc.scalar, xtq_sb, xTq, 1)

            # ---- V projection: V[own rows, o] --------------------------------
            vdst = vstg if version == 2 else v_sb
            for st_i in range(KH // P):
                for oh in range(2):
                    ps = pspool.tile([P, 512], F32, tag="mm", bufs=3)
                    for d in range(NDC):
                        nc.tensor.matmul(
                            ps[:],
                            xt_sb[:, st_i // 4, d, (st_i % 4) * P : (st_i % 4 + 1) * P],
                            wv_sb[:, oh, d, :],
                            start=(d == 0),
                            stop=(d == NDC - 1),
                        )
                    copy_out(vdst[:, st_i, oh * 512 : (oh + 1) * 512], ps[:])
                if version == 2 and st_i == 3:
                    # first V half staged -> gather it while the second half
                    # computes; kt readback rides the gpsimd queue in between
                    # (K gather already done, so it doesn't block the V gather).
                    # Gather A rows = [s 0:512 | s 1024:1536] (rank-major).
                    nc.sync.dma_start(
                        vag_in[0:512, :].rearrange("(t p) o -> p t o", p=P),
                        vstg[:, 0:4, :],
                    )
                    nc.gpsimd.collective_compute(
                        "AllGather", mybir.AluOpType.bypass, replica_groups=cc,
                        ins=[vag_in[0:512, :]], outs=[vag_out[0:1024, :]],
                    )
                    # kt readback on gpsimd behind gather A (K gather long
                    # done); keeping collective-dependent waits off the sync
                    # queue — a sync wait on an unfinished collective deadlocks
                    for r in range(2):
                        nc.gpsimd.dma_start(
                            kt_sb[:, :, r * KH : (r + 1) * KH],
                            ktag_out[r * D : (r + 1) * D, :].rearrange(
                                "(c p) k -> p c k", p=P
                            ),
                        )
                    # vagA available: v s-tiles 0-3 and 8-11
                    nc.gpsimd.dma_start(
                        v_sb[:, 0:4, :],
                        vag_out[0:512, :].rearrange("(t p) o -> p t o", p=P),
                    )
                    nc.gpsimd.dma_start(
                        v_sb[:, 8:12, :],
                        vag_out[512:1024, :].rearrange("(t p) o -> p t o", p=P),
                    )
            if version == 2:
                # Gather B rows = [s 512:1024 | s 1536:2048]
                nc.sync.dma_start(
                    vag_in[512:1024, :].rearrange("(t p) o -> p t o", p=P),
                    vstg[:, 4:8, :],
                )
                nc.gpsimd.collective_compute(
                    "AllGather", mybir.AluOpType.bypass, replica_groups=cc,
                    ins=[vag_in[512:1024, :]], outs=[vag_out[1024:2048, :]],
                )

            # ---- Q projection: QT[o, q] --------------------------------------
            for qs in range(QCORE // 512):
                for c in range(NDC):
                    ps = pspool.tile([P, 512], F32, tag="mm", bufs=3)
                    for d in range(NDC):
                        nc.tensor.matmul(
                            ps[:],
                            wq_sb[:, c // 4, d, (c % 4) * P : (c % 4 + 1) * P],
                            xtq_sb[:, qs, d, :],
                            start=(d == 0),
                            stop=(d == NDC - 1),
                        )
                    copy_out(qt_sb[:, c, qs * 512 : (qs + 1) * 512], ps[:])

            if version == 2:
                # vagB readback: v s-tiles 4-7 first (needed by PV from j=2 on)
                nc.gpsimd.dma_start(
                    v_sb[:, 4:8, :],
                    vag_out[1024:1536, :].rearrange("(t p) o -> p t o", p=P),
                )
                nc.gpsimd.dma_start(
                    v_sb[:, 12:16, :],
                    vag_out[1536:2048, :].rearrange("(t p) o -> p t o", p=P),
                )

    # ---- attention, software-pipelined per q-tile position ---------------
    with tc.tile_pool(name="sp", bufs=2) as sp, tc.tile_pool(
        name="pp", bufs=2
    ) as pp, tc.tile_pool(name="stats", bufs=4) as stp, tc.tile_pool(
        name="atp", bufs=4
    ) as atp, tc.tile_pool(name="cp", bufs=2) as cp:
        state = {}

        def emit_scores(j):
            L = LJS[j]
            ssb = sp.tile([P, L], F32, tag="ssb", name=f"ssb{j}")
            for off, w, is_mask in _score_chunks(L):
                ps = pspool.tile([P, 512], F32, tag="mm", bufs=3)
                for c in range(NDC):
                    nc.tensor.matmul(
                        ps[:, :w],
                        qt_sb[:, c, j * P : (j + 1) * P],
                        kt_sb[:, c, off : off + w],
                        start=(c == 0),
                        stop=(c == NDC - 1),
                    )
                if is_mask:
                    nc.vector.tensor_add(
                        ssb[:, off : off + w], ps[:, :w], masksb[:, j, :]
                    )
                else:
                    copy_out(ssb[:, off : off + w], ps[:, :w])
            state[j] = ssb

        def emit_softmax_pv(j):
            L = LJS[j]
            ssb = state.pop(j)
            psb = pp.tile([P, L], BF16, tag="psb", name=f"psb{j}")
            sumv = stp.tile([P, 1], F32, tag="sumv", name=f"sumv{j}")
            nc.scalar.activation(
                psb[:],
                ssb[:],
                mybir.ActivationFunctionType.Exp,
                scale=1.0,
                accum_out=sumv[:],
            )
            rcp = stp.tile([P, 1], F32, tag="rcp", name=f"rcp{j}")
            nc.vector.reciprocal(rcp[:], sumv[:])

            co0 = pspool.tile([P, 512], F32, tag="co", bufs=2, name=f"co0_{j}")
            co1 = pspool.tile([P, 512], F32, tag="co", bufs=2, name=f"co1_{j}")
            nkt = L // P
            for k in range(nkt):
                tp = pspool.tile([P, P], BF16, tag="tp", bufs=3, name=f"tp{j}_{k}")
                nc.tensor.transpose(tp[:], psb[:, k * P : (k + 1) * P], identsb[:])
                at = atp.tile([P, P], BF16, tag="at", name=f"at{j}_{k}")
                copy_out(at[:], tp[:])
                nc.tensor.matmul(
                    co0[:], at[:], v_sb[:, k, 0:512],
                    start=(k == 0), stop=(k == nkt - 1),
                )
                nc.tensor.matmul(
                    co1[:], at[:], v_sb[:, k, 512:1024],
                    start=(k == 0), stop=(k == nkt - 1),
                )
            csb = cp.tile([P, D], F32, tag="csb", name=f"csb{j}")
            nc.vector.tensor_scalar_mul(csb[:, 0:512], co0[:], rcp[:])
            nc.sync.dma_start(out[j * P : (j + 1) * P, 0:512], csb[:, 0:512])
            nc.scalar.activation(
                csb[:, 512:1024],
                co1[:],
                mybir.ActivationFunctionType.Copy,
                scale=rcp[:],
            )
            nc.sync.dma_start(out[j * P : (j + 1) * P, 512:1024], csb[:, 512:1024])

        emit_scores(0)
        for j in range(1, NQT):
            emit_scores(j)
            emit_softmax_pv(j - 1)
        emit_softmax_pv(NQT - 1)


def _emit_body_v45(nc, tc, rctx, aps, version=4):
    """S^T-direct attention (no PE transposes). version 4: V projected
    locally for all 2048 rows (duplicated per pair, like v3). version 5:
    V projection sharded by rows + pairwise AllGather (like v2).

    Attention works on two quads of 4 sorted q-tile positions each. For
    quad g (positions p = 4g..4g+3), scores are computed TRANSPOSED:
      S^T[k, q] = sum_o KT[o, k-block] QT[o, q]   (k on partitions)
    per 128-row k-block kb, over the q columns of all members that
    causally need kb (member p needs kb <= 2p+1 across both h cores; the
    h-specific overhang + diagonal triangle are handled by a per-core
    maskT input added on DVE). exp(S^T) lands directly in the PV
    stationary layout, so the 72 PE transposes + PSUM->SBUF P^T copies
    of v3 disappear. The softmax row-sum rides a 1-column ones-matmul
    that reuses the already-loaded P^T stationary.
    """
    if version == 4:
        (xT, xTk, xTq, wqT, wkT, wvT, masksT, ones, out,
         ktag_in, ktag_out, pspool) = aps
    else:
        (xT, xTq, wqT, wkT, wvT, masksT, ones, out,
         ktag_in, ktag_out, vag_in, vag_out, pspool) = aps
    cc = [[0, 1], [2, 3], [4, 5], [6, 7]]
    KH = S if version == 4 else S // 2   # V rows projected locally
    KK = S // 2                          # K rows projected locally
    copy_ctr = [0]

    def copy_out(dst, src):
        copy_ctr[0] += 1
        if copy_ctr[0] % 2:
            nc.vector.tensor_copy(dst, src)
        else:
            nc.scalar.copy(dst, src)

    ktpool = rctx.enter_context(tc.tile_pool(name="ktp", bufs=1))
    # KT chunk-major: [o%128, key-chunk(512), o//128, k%512] so each
    # gather-readback DMA writes one contiguous chunk (no interleaved
    # address ranges -> no false deps stalling the first scores)
    kt_sb = ktpool.tile([P, 4, NDC, 512], BF16)
    vpool = rctx.enter_context(tc.tile_pool(name="vp", bufs=1))
    v_sb = vpool.tile([P, S // P, D], BF16)    # V: [s%128, s//128, o]
    qtpool = rctx.enter_context(tc.tile_pool(name="qtp", bufs=1))
    qt_sb = qtpool.tile([P, NDC, QCORE], BF16)  # QT: [o%128, o//128, q]

    with tc.tile_pool(name="wts", bufs=1) as wpool:
        wv_sb = wpool.tile([P, 2, NDC, 512], BF16)
        wq_sb = wpool.tile([P, 2, NDC, 512], BF16)
        xtq_sb = wpool.tile([P, 2, NDC, 512], BF16)
        xt_sb = wpool.tile([P, KH // 512, NDC, 512], BF16)

        def ld(eng, dst, w, g):
            eng.dma_start(dst[:, g], w[:, g])

        def ld2(eng_a, eng_b, dst, w, g):
            # split one 512-col group across two DMA queues (halves the
            # startup-critical load latency); only sync/scalar/gpsimd
            # have DMA queues
            eng_a.dma_start(dst[:, g, 0:4], w[:, g, 0:4])
            eng_b.dma_start(dst[:, g, 4:8], w[:, g, 4:8])

        # ---- K phase ----------------------------------------------------
        with tc.tile_pool(name="kph", bufs=1) as kpool:
            wk_sb = kpool.tile([P, 2, NDC, 512], BF16)
            if version == 4:
                xtk_sb = kpool.tile([P, KK // 512, NDC, 512], BF16)
                xk_src = xTk
            else:
                xtk_sb = xt_sb
                xk_src = xT

            # DMA issuance lives in the issuing ENGINE's instruction
            # FIFO with flow-control waits, so loads must NEVER ride the
            # scalar/vector engines (they do the PSUM copy-outs — a
            # queued load blocks them and stalls the PE). All input
            # loads go on sync + gpsimd only, in first-use order; the
            # startup-critical wk/x g0 are split across both queues.
            # gpsimd carries nothing after xtk g1 (reserved for K
            # staging + collective + readbacks).
            def ldp(eng, dst, w, g, d0, d1):
                eng.dma_start(dst[:, g, d0:d1], w[:, g, d0:d1])

            ldp(nc.sync, wk_sb, wkT, 0, 0, 4)
            ldp(nc.gpsimd, wk_sb, wkT, 0, 4, 8)
            ldp(nc.sync, xtk_sb, xk_src, 0, 0, 4)
            ldp(nc.gpsimd, xtk_sb, xk_src, 0, 4, 8)
            ld(nc.sync, wk_sb, wkT, 1)
            if version == 4:
                ld(nc.gpsimd, xtk_sb, xTk, 1)
                rest = [(wv_sb, wvT, 0), (wv_sb, wvT, 1),
                        (xt_sb, xT, 0), (xt_sb, xT, 1),
                        (xt_sb, xT, 2), (xt_sb, xT, 3)]
            else:
                ld(nc.gpsimd, xt_sb, xT, 1)
                rest = [(wv_sb, wvT, 0), (wv_sb, wvT, 1)]
            rest += [(xtq_sb, xTq, 0), (xtq_sb, xTq, 1),
                     (wq_sb, wqT, 0), (wq_sb, wqT, 1)]
            for dst, w, g in rest:
                ld(nc.sync, dst, w, g)

            # K projection: KT[o, own keys]; stage per 4-c half (512KB
            # DMAs: descriptor generation on the gpsimd engine costs
            # ~1us per DMA, so fewer+bigger beats 16x128KB) with a
            # 3-buffered half-size staging tile so the next group's
            # copy-outs never wait on a staging drain
            for ks in range(KK // 512):
                for ch in range(2):
                    kst = kpool.tile(
                        [P, 4, 512], BF16, tag="kstg", bufs=3,
                        name=f"kst{ks}_{ch}",
                    )
                    for c4 in range(4):
                        c = ch * 4 + c4
                        ps = pspool.tile([P, 512], F32, tag="mm", bufs=3)
                        for d in range(NDC):
                            nc.tensor.matmul(
                                ps[:],
                                wk_sb[:, c // 4, d, (c % 4) * P : (c % 4 + 1) * P],
                                xtk_sb[:, ks, d, :],
                                start=(d == 0),
                                stop=(d == NDC - 1),
                            )
                        copy_out(kst[:, c4, :], ps[:])
                    nc.gpsimd.dma_start(
                        ktag_in[
                            ch * 512 : (ch + 1) * 512,
                            ks * 512 : (ks + 1) * 512,
                        ].rearrange("(c p) k -> p c k", p=P),
                        kst[:],
                    )
            nc.gpsimd.collective_compute(
                "AllGather", mybir.AluOpType.bypass, replica_groups=cc,
                ins=[ktag_in[:]], outs=[ktag_out[:]],
            )
            # readback in 4 chunked DMAs; chunk ci = global keys
            # ci*512:(ci+1)*512 = ktag_out[rank(ci)*D rows, (ci%2) cols]
            for ci in range(4):
                r, half = ci // 2, ci % 2
                nc.gpsimd.dma_start(
                    kt_sb[:, ci],
                    ktag_out[
                        r * D : (r + 1) * D, half * 512 : (half + 1) * 512
                    ].rearrange("(c p) k -> p c k", p=P),
                )

        # ---- V + Q projections ------------------------------------------
        with tc.tile_pool(name="qph", bufs=1) as qpool:
            if version == 5:
                vstg = qpool.tile([P, KH // P, D], BF16)

            vdst = vstg if version == 5 else v_sb
            for st_i in range(KH // P):
                for oh in range(2):
                    ps = pspool.tile([P, 512], F32, tag="mm", bufs=3)
                    for d in range(NDC):
                        nc.tensor.matmul(
                            ps[:],
                            xt_sb[:, st_i // 4, d, (st_i % 4) * P : (st_i % 4 + 1) * P],
                            wv_sb[:, oh, d, :],
                            start=(d == 0),
                            stop=(d == NDC - 1),
                        )
                    copy_out(vdst[:, st_i, oh * 512 : (oh + 1) * 512], ps[:])
                if version == 5 and st_i == 3:
                    nc.sync.dma_start(
                        vag_in[0:512, :].rearrange("(t p) o -> p t o", p=P),
                        vstg[:, 0:4, :],
                    )
                    nc.gpsimd.collective_compute(
                        "AllGather", mybir.AluOpType.bypass, replica_groups=cc,
                        ins=[vag_in[0:512, :]], outs=[vag_out[0:1024, :]],
                    )
                    # gather A yields global s-tiles 0-3 (rank0) + 8-11
                    nc.gpsimd.dma_start(
                        v_sb[:, 0:4, :],
                        vag_out[0:512, :].rearrange("(t p) o -> p t o", p=P),
                    )
                    nc.gpsimd.dma_start(
                        v_sb[:, 8:12, :],
                        vag_out[512:1024, :].rearrange("(t p) o -> p t o", p=P),
                    )
            if version == 5:
                nc.sync.dma_start(
                    vag_in[512:1024, :].rearrange("(t p) o -> p t o", p=P),
                    vstg[:, 4:8, :],
                )
                nc.gpsimd.collective_compute(
                    "AllGather", mybir.AluOpType.bypass, replica_groups=cc,
                    ins=[vag_in[512:1024, :]], outs=[vag_out[1024:2048, :]],
                )
                nc.gpsimd.dma_start(
                    v_sb[:, 4:8, :],
                    vag_out[1024:1536, :].rearrange("(t p) o -> p t o", p=P),
                )
                nc.gpsimd.dma_start(
                    v_sb[:, 12:16, :],
                    vag_out[1536:2048, :].rearrange("(t p) o -> p t o", p=P),
                )

            # Q projection: QT[o, q]
            for qs in range(QCORE // 512):
                for c in range(NDC):
                    ps = pspool.tile([P, 512], F32, tag="mm", bufs=3)
                    for d in range(NDC):
                        nc.tensor.matmul(
                            ps[:],
                            wq_sb[:, c // 4, d, (c % 4) * P : (c % 4 + 1) * P],
                            xtq_sb[:, qs, d, :],
                            start=(d == 0),
                            stop=(d == NDC - 1),
                        )
                    copy_out(qt_sb[:, c, qs * 512 : (qs + 1) * 512], ps[:])

    # ---- attention: S^T-direct over two quads ----------------------------
    with tc.tile_pool(name="ptq", bufs=1) as ptpool, tc.tile_pool(
        name="stats", bufs=4
    ) as stp, tc.tile_pool(name="cp", bufs=2) as cp, tc.tile_pool(
        name="accp", bufs=1
    ) as accp:
        onesb = accp.tile([P, 1], F32)
        maskTsb = accp.tile([P, NQT, 2, P], F32)
        nc.sync.dma_start(onesb[:], ones[:])
        nc.sync.dma_start(maskTsb[:], masksT[:])
        pt = {
            g: ptpool.tile(
                [P, 8 * g + 8, 512], BF16, tag=f"ptq{g}", name=f"ptq{g}"
            )
            for g in (0, 1)
        }
        # per-quad f32 rowsum accumulators [k-part, member q cols]; the
        # partial-width adds ([qoff:512]) accumulate exactly each
        # member's causal kb range
        acc = {
            g: accp.tile([P, 512], F32, tag=f"acc{g}", name=f"acc{g}")
            for g in (0, 1)
        }

        def emit_scores_st(g, kbs):
            for kb in kbs:
                p_lo = max(4 * g, kb // 2)
                qoff = (p_lo - 4 * g) * P
                st = pspool.tile([P, 512], F32, tag="mm", bufs=3)
                for c in range(NDC):
                    nc.tensor.matmul(
                        st[:, qoff:512],
                        kt_sb[:, kb // 4, c, (kb % 4) * P : (kb % 4 + 1) * P],
                        qt_sb[:, c, g * 512 + qoff : (g + 1) * 512],
                        start=(c == 0),
                        stop=(c == NDC - 1),
                    )
                pm = kb // 2  # member whose diagonal/overhang region this is
                if pm >= 4 * g:
                    mo = (pm - 4 * g) * P
                    nc.vector.tensor_add(
                        st[:, mo : mo + P], st[:, mo : mo + P],
                        maskTsb[:, pm, kb % 2, :],
                    )
                nc.scalar.activation(
                    pt[g][:, kb, qoff:512],
                    st[:, qoff:512],
                    mybir.ActivationFunctionType.Exp,
                    scale=1.0,
                )
                if kb == 0:
                    nc.vector.tensor_copy(acc[g][:, 0:512], pt[g][:, 0, 0:512])
                else:
                    nc.vector.tensor_add(
                        acc[g][:, qoff:512], acc[g][:, qoff:512],
                        pt[g][:, kb, qoff:512],
                    )

        def emit_pv(g, members):
            for p in members:
                qo = (p - 4 * g) * P
                co0 = pspool.tile([P, 512], F32, tag="co", bufs=4, name=f"co0_{p}")
                co1 = pspool.tile([P, 512], F32, tag="co", bufs=4, name=f"co1_{p}")
                last = 2 * p + 1
                for kb in range(last + 1):
                    lhsT = pt[g][:, kb, qo : qo + P]
                    nc.tensor.matmul(co0[:], lhsT, v_sb[:, kb, 0:512],
                                     start=(kb == 0), stop=(kb == last))
                    nc.tensor.matmul(co1[:], lhsT, v_sb[:, kb, 512:1024],
                                     start=(kb == 0), stop=(kb == last))
                # cross-partition rowsum: ones-contraction of the f32
                # accumulator chunk (q lands on partitions)
                rs = pspool.tile([P, 1], F32, tag="rs", bufs=1, name=f"rs{p}")
                nc.tensor.matmul(rs[:], acc[g][:, qo : qo + P], onesb[:],
                                 start=True, stop=True)
                rcp = stp.tile([P, 1], F32, tag="rcp", name=f"rcp{p}")
                nc.vector.reciprocal(rcp[:], rs[:])
                csb = cp.tile([P, D], F32, tag="csb", name=f"csb{p}")
                for ch in range(4):
                    lo, hi = ch * 256, (ch + 1) * 256
                    src = co0 if ch < 2 else co1
                    slo, shi = (lo, hi) if ch < 2 else (lo - 512, hi - 512)
                    if ch % 2 == 0:
                        nc.vector.tensor_scalar_mul(
                            csb[:, lo:hi], src[:, slo:shi], rcp[:]
                        )
                    else:
                        nc.scalar.activation(
                            csb[:, lo:hi], src[:, slo:shi],
                            mybir.ActivationFunctionType.Copy, scale=rcp[:],
                        )
                    nc.sync.dma_start(
                        out[p * P : (p + 1) * P, lo:hi], csb[:, lo:hi]
                    )

        # order: quad0 scores, quad1 scores (first half), quad0 PV m0/m1,
        # quad1 scores (rest), quad0 PV m2/m3, quad1 PV — PE stays dense
        # while the K/V gathers get maximum slack
        emit_scores_st(0, range(8))
        emit_scores_st(1, range(8))
        emit_pv(0, (0, 1))
        emit_scores_st(1, range(8, 16))
        emit_pv(0, (2, 3))
        emit_pv(1, (4, 5, 6, 7))


def _load_idx(eng, ap, name, mx):
    tmp = eng.alloc_register(name)
    eng.reg_load(tmp, ap[0:1, 0:1])
    return eng.snap(tmp, donate=True, min_val=0, max_val=mx)


def _emit_body_v6(nc, tc, rctx, aps):
    """K and V projections sharded across each HBM pair, exchanged via
    pair-shared HBM (addr_space='Shared') with plain DMAs at full
    bandwidth. Collectives are used only as tiny barriers (the 62GB/s
    collective data path + its serialization killed v5). Per-core
    index inputs (h, 1-h, 2h, 2h+1) drive bass.ds dynamic slices so
    the SPMD program stays identical across cores.
    """
    (xT, xTq, wqT, wkT, wvT, masksT, ones, hsel, prd, koff0, koff1,
     out, kxch, vxch, b1i, b1o, b2i, b2o, pspool) = aps
    cc = [[0, 1], [2, 3], [4, 5], [6, 7]]
    KK = S // 2
    copy_ctr = [0]

    # per-engine index registers (engine registers are private)
    hs_gp = _load_idx(nc.gpsimd, hsel, "hs_gp", 1)
    prd_gp = _load_idx(nc.gpsimd, prd, "prd_gp", 1)
    hs_sy = _load_idx(nc.sync, hsel, "hs_sy", 1)
    koff_v = [_load_idx(nc.vector, koff0, "k0v", 3),
              _load_idx(nc.vector, koff1, "k1v", 3)]
    koff_s = [_load_idx(nc.scalar, koff0, "k0s", 3),
              _load_idx(nc.scalar, koff1, "k1s", 3)]
    hs_v = _load_idx(nc.vector, hsel, "hs_v", 1)
    hs_s = _load_idx(nc.scalar, hsel, "hs_s", 1)
    koff_gp = [_load_idx(nc.gpsimd, koff0, "k0g", 3),
               _load_idx(nc.gpsimd, koff1, "k1g", 3)]

    def copy_out(dst_pair, src):
        # dst_pair = (vector_dst, scalar_dst) - dynamic dsts need the
        # issuing engine's own register
        copy_ctr[0] += 1
        if copy_ctr[0] % 2:
            nc.vector.tensor_copy(dst_pair[0], src)
        else:
            nc.scalar.copy(dst_pair[1], src)

    ktpool = rctx.enter_context(tc.tile_pool(name="ktp", bufs=1))
    kt_sb = ktpool.tile([P, 4, NDC, 512], BF16)  # key-chunk-major KT
    vpool = rctx.enter_context(tc.tile_pool(name="vp", bufs=1))
    v_sb = vpool.tile([P, S // P, D], BF16)
    qtpool = rctx.enter_context(tc.tile_pool(name="qtp", bufs=1))
    qt_sb = qtpool.tile([P, NDC, QCORE], BF16)

    with tc.tile_pool(name="wts", bufs=1) as wpool:
        wv_sb = wpool.tile([P, 2, NDC, 512], BF16)
        wq_sb = wpool.tile([P, 2, NDC, 512], BF16)
        xtq_sb = wpool.tile([P, 2, NDC, 512], BF16)
        xt_sb = wpool.tile([P, 2, NDC, 512], BF16)  # own-half x

        def ld(eng, dst, w, g):
            eng.dma_start(dst[:, g], w[:, g])

        def ldp(eng, dst, w, g, d0, d1):
            eng.dma_start(dst[:, g, d0:d1], w[:, g, d0:d1])

        with tc.tile_pool(name="kph", bufs=1) as kpool:
            wk_sb = kpool.tile([P, 2, NDC, 512], BF16)

            ldp(nc.sync, wk_sb, wkT, 0, 0, 4)
            ldp(nc.gpsimd, wk_sb, wkT, 0, 4, 8)
            ldp(nc.sync, xt_sb, xT, 0, 0, 4)
            ldp(nc.gpsimd, xt_sb, xT, 0, 4, 8)
            ld(nc.sync, wk_sb, wkT, 1)
            ld(nc.gpsimd, xt_sb, xT, 1)
            for dst, w, g in [(wv_sb, wvT, 0), (wv_sb, wvT, 1),
                              (xtq_sb, xTq, 0), (xtq_sb, xTq, 1),
                              (wq_sb, wqT, 0), (wq_sb, wqT, 1)]:
                ld(nc.sync, dst, w, g)

            # K projection: own keys -> kt_sb at dynamic chunk 2h+ks,
            # staged to pair-shared kxch[h]
            for ks in range(KK // 512):
                for ch in range(2):
                    for c4 in range(4):
                        c = ch * 4 + c4
                        ps = pspool.tile([P, 512], F32, tag="mm", bufs=3)
                        for d in range(NDC):
                            nc.tensor.matmul(
                                ps[:],
                                wk_sb[:, c // 4, d, (c % 4) * P : (c % 4 + 1) * P],
                                xt_sb[:, ks, d, :],
                                start=(d == 0),
                                stop=(d == NDC - 1),
                            )
                        src = ps[:].rearrange("p (a k) -> p a k", a=1)
                        copy_out(
                            (kt_sb[:, bass.ds(koff_v[ks], 1), c, :],
                             kt_sb[:, bass.ds(koff_s[ks], 1), c, :]),
                            src,
                        )
                    nc.gpsimd.dma_start(
                        kxch[bass.ds(hs_gp, 1), ks, ch].rearrange(
                            "r p c k -> p r c k"
                        ),
                        kt_sb[:, bass.ds(koff_gp[ks], 1), ch * 4 : (ch + 1) * 4, :],
                    )
            # barrier 1: completion implies the pair partner's K staging
            # (it enters only after its own stagings) is visible
            nc.gpsimd.dma_start(
                b1i[:], kxch[bass.ds(hs_gp, 1), 0, 0, 0:1, 0:1, 0:4]
            )
            nc.gpsimd.collective_compute(
                "AllGather", mybir.AluOpType.bypass, replica_groups=cc,
                ins=[b1i[:]], outs=[b1o[:]],
            )
            # land the barrier output in kxch's scratch slice (ks=2):
            # the readbacks read kxch with dynamic offsets, so Tile's
            # conservative dep tracking makes them wait this write —
            # i.e. the barrier — before issuing (a plain engine-order
            # assumption raced on hardware)
            nc.gpsimd.dma_start(
                kxch[bass.ds(hs_gp, 1), 2, 0, 0, 0, 0:4], b1o[0:1, :]
            )
            # partner KT readback (full-bandwidth plain DMA)
            ktv = kt_sb[:].rearrange("p (r s) c k -> p r s c k", r=2)
            for ks in range(2):
                for ch in range(2):
                    nc.gpsimd.dma_start(
                        ktv[:, bass.ds(prd_gp, 1), ks, ch * 4 : (ch + 1) * 4, :],
                        kxch[bass.ds(prd_gp, 1), ks, ch].rearrange(
                            "r p c k -> p r c k"
                        ),
                    )

        # ---- V (own half) + Q projections ---------------------------
        v_sbv = v_sb[:].rearrange("p (r t) o -> p r t o", r=2)
        for st_i in range(8):
            for oh in range(2):
                ps = pspool.tile([P, 512], F32, tag="mm", bufs=3)
                for d in range(NDC):
                    nc.tensor.matmul(
                        ps[:],
                        xt_sb[:, st_i // 4, d, (st_i % 4) * P : (st_i % 4 + 1) * P],
                        wv_sb[:, oh, d, :],
                        start=(d == 0),
                        stop=(d == NDC - 1),
                    )
                src = ps[:].rearrange("p (a k) -> p a k", a=1)
                copy_out(
                    (v_sbv[:, bass.ds(hs_v, 1), st_i, oh * 512 : (oh + 1) * 512],
                     v_sbv[:, bass.ds(hs_s, 1), st_i, oh * 512 : (oh + 1) * 512]),
                    src,
                )
        # stage own V half to pair-shared vxch[h] (sync queue)
        nc.sync.dma_start(
            vxch[bass.ds(hs_sy, 1), 0:8].rearrange("r t p o -> p r t o"),
            v_sbv[:, bass.ds(hs_sy, 1)],
        )
        # barrier 2 + partner V readback
        nc.gpsimd.dma_start(
            b2i[:], vxch[bass.ds(hs_gp, 1), 0, 0:1, 0:4]
        )
        nc.gpsimd.collective_compute(
            "AllGather", mybir.AluOpType.bypass, replica_groups=cc,
            ins=[b2i[:]], outs=[b2o[:]],
        )
        # barrier-2 output into vxch scratch (tile 8) for the same
        # conservative-dep ordering of the V readback
        nc.gpsimd.dma_start(
            vxch[bass.ds(hs_gp, 1), 8, 0, 0:4], b2o[0:1, :]
        )
        nc.gpsimd.dma_start(
            v_sbv[:, bass.ds(prd_gp, 1)],
            vxch[bass.ds(prd_gp, 1), 0:8].rearrange("r t p o -> p r t o"),
        )

        # Q projection
        for qs in range(QCORE // 512):
            for c in range(NDC):
                ps = pspool.tile([P, 512], F32, tag="mm", bufs=3)
                for d in range(NDC):
                    nc.tensor.matmul(
                        ps[:],
                        wq_sb[:, c // 4, d, (c % 4) * P : (c % 4 + 1) * P],
                        xtq_sb[:, qs, d, :],
                        start=(d == 0),
                        stop=(d == NDC - 1),
                    )
                dst = qt_sb[:, c, qs * 512 : (qs + 1) * 512]
                copy_out((dst, dst), ps[:])

    _emit_attention_st(nc, tc, pspool, kt_sb, qt_sb, v_sb, masksT, ones, out)


def _emit_attention_st(nc, tc, pspool, kt_sb, qt_sb, v_sb, masksT, ones, out):
    with tc.tile_pool(name="ptq", bufs=1) as ptpool, tc.tile_pool(
        name="stats", bufs=4
    ) as stp, tc.tile_pool(name="cp", bufs=2) as cp, tc.tile_pool(
        name="accp", bufs=1
    ) as accp:
        onesb = accp.tile([P, 1], F32)
        maskTsb = accp.tile([P, NQT, 2, P], F32)
        nc.sync.dma_start(onesb[:], ones[:])
        nc.sync.dma_start(maskTsb[:], masksT[:])
        pt = {
            g: ptpool.tile(
                [P, 8 * g + 8, 512], BF16, tag=f"ptq{g}", name=f"ptq{g}"
            )
            for g in (0, 1)
        }
        acc = {
            g: accp.tile([P, 512], F32, tag=f"acc{g}", name=f"acc{g}")
            for g in (0, 1)
        }

        def emit_scores_st(g, kbs):
            for kb in kbs:
                p_lo = max(4 * g, kb // 2)
                qoff = (p_lo - 4 * g) * P
                st = pspool.tile([P, 512], F32, tag="mm", bufs=3)
                for c in range(NDC):
                    nc.tensor.matmul(
                        st[:, qoff:512],
                        kt_sb[:, kb // 4, c, (kb % 4) * P : (kb % 4 + 1) * P],
                        qt_sb[:, c, g * 512 + qoff : (g + 1) * 512],
                        start=(c == 0),
                        stop=(c == NDC - 1),
                    )
                pm = kb // 2
                if pm >= 4 * g:
                    mo = (pm - 4 * g) * P
                    nc.vector.tensor_add(
                        st[:, mo : mo + P], st[:, mo : mo + P],
                        maskTsb[:, pm, kb % 2, :],
                    )
                nc.scalar.activation(
                    pt[g][:, kb, qoff:512],
                    st[:, qoff:512],
                    mybir.ActivationFunctionType.Exp,
                    scale=1.0,
                )
                if kb == 0:
                    nc.vector.tensor_copy(acc[g][:, 0:512], pt[g][:, 0, 0:512])
                else:
                    nc.vector.tensor_add(
                        acc[g][:, qoff:512], acc[g][:, qoff:512],
                        pt[g][:, kb, qoff:512],
                    )

        def emit_pv(g, members):
            for p in members:
                qo = (p - 4 * g) * P
                co0 = pspool.tile([P, 512], F32, tag="co", bufs=4, name=f"co0_{p}")
                co1 = pspool.tile([P, 512], F32, tag="co", bufs=4, name=f"co1_{p}")
                last = 2 * p + 1
                for kb in range(last + 1):
                    lhsT = pt[g][:, kb, qo : qo + P]
                    nc.tensor.matmul(co0[:], lhsT, v_sb[:, kb, 0:512],
                                     start=(kb == 0), stop=(kb == last))
                    nc.tensor.matmul(co1[:], lhsT, v_sb[:, kb, 512:1024],
                                     start=(kb == 0), stop=(kb == last))
                rs = pspool.tile([P, 1], F32, tag="rs", bufs=1, name=f"rs{p}")
                nc.tensor.matmul(rs[:], acc[g][:, qo : qo + P], onesb[:],
                                 start=True, stop=True)
                rcp = stp.tile([P, 1], F32, tag="rcp", name=f"rcp{p}")
                nc.vector.reciprocal(rcp[:], rs[:])
                csb = cp.tile([P, D], F32, tag="csb", name=f"csb{p}")
                for ch in range(4):
                    lo, hi = ch * 256, (ch + 1) * 256
                    src = co0 if ch < 2 else co1
                    slo, shi = (lo, hi) if ch < 2 else (lo - 512, hi - 512)
                    if ch % 2 == 0:
                        nc.vector.tensor_scalar_mul(
                            csb[:, lo:hi], src[:, slo:shi], rcp[:]
                        )
                    else:
                        nc.scalar.activation(
                            csb[:, lo:hi], src[:, slo:shi],
                            mybir.ActivationFunctionType.Copy, scale=rcp[:],
                        )
                    nc.sync.dma_start(
                        out[p * P : (p + 1) * P, lo:hi], csb[:, lo:hi]
                    )

        emit_scores_st(0, range(8))
        emit_scores_st(1, range(8))
        emit_pv(0, (0, 1))
        emit_scores_st(1, range(8, 16))
        emit_pv(0, (2, 3))
        emit_pv(1, (4, 5, 6, 7))


def _build(version=1):
    nc = bacc.Bacc("TRN2", target_bir_lowering=False, debug=False, num_devices=8)

    if version == 6:
        def packed6(name, cols):
            return nc.dram_tensor(
                name, [P, cols // 512, NDC, 512], BF16, kind="ExternalInput"
            ).ap()

        def idx_in(name):
            return nc.dram_tensor(
                name, [1, 1], mybir.dt.uint32, kind="ExternalInput"
            ).ap()

        xT = packed6("xT", S // 2)
        xTq = packed6("xTq", QCORE)
        wqT = packed6("wqT", D)
        wkT = packed6("wkT", D)
        wvT = packed6("wvT", D)
        masksT = nc.dram_tensor(
            "masksT", [P, NQT, 2, P], F32, kind="ExternalInput"
        ).ap()
        ones = nc.dram_tensor("ones", [P, 1], F32, kind="ExternalInput").ap()
        hsel, prd = idx_in("hsel"), idx_in("prd")
        koff0, koff1 = idx_in("koff0"), idx_in("koff1")
        out = nc.dram_tensor("out", [QCORE, D], F32, kind="ExternalOutput").ap()
        kxch = nc.dram_tensor(
            "kxch", [2, 3, 2, P, 4, 512], BF16, addr_space="Shared"
        ).ap()
        vxch = nc.dram_tensor(
            "vxch", [2, 9, P, D], BF16, addr_space="Shared"
        ).ap()
        b1i = nc.dram_tensor("b1i", [1, 4], BF16).ap()
        b1o = nc.dram_tensor("b1o", [2, 4], BF16).ap()
        b2i = nc.dram_tensor("b2i", [1, 4], BF16).ap()
        b2o = nc.dram_tensor("b2o", [2, 4], BF16).ap()
        with tile.TileContext(nc) as tc, ExitStack() as rctx:
            pspool = rctx.enter_context(
                tc.tile_pool(name="ps", bufs=2, space=bass.MemorySpace.PSUM)
            )
            _emit_body_v6(
                nc, tc, rctx,
                (xT, xTq, wqT, wkT, wvT, masksT, ones, hsel, prd, koff0,
                 koff1, out, kxch, vxch, b1i, b1o, b2i, b2o, pspool),
            )
        nc.compile()
        return nc

    kh = S if version in (1, 3, 4) else S // 2

    def packed(name, cols):
        # host-prepacked [128, cols//512, 8, 512] input (SBUF tile layout)
        return nc.dram_tensor(
            name, [P, cols // 512, NDC, 512], BF16, kind="ExternalInput"
        ).ap()

    xT = packed("xT", kh)
    xTq = packed("xTq", QCORE)
    wqT = packed("wqT", D)
    wkT = packed("wkT", D)
    wvT = packed("wvT", D)
    out = nc.dram_tensor("out", [QCORE, D], F32, kind="ExternalOutput").ap()
    if version in (4, 5):
        masksT = nc.dram_tensor(
            "masksT", [P, NQT, 2, P], F32, kind="ExternalInput"
        ).ap()
        ones = nc.dram_tensor("ones", [P, 1], F32, kind="ExternalInput").ap()
        ktag_in = nc.dram_tensor("ktag_in", [D, S // 2], BF16).ap()
        ktag_out = nc.dram_tensor("ktag_out", [2 * D, S // 2], BF16).ap()
        if version == 4:
            xTk = packed("xTk", S // 2)
            aps_head = (xT, xTk, xTq, wqT, wkT, wvT, masksT, ones, out,
                        ktag_in, ktag_out)
        else:
            vag_in = nc.dram_tensor("vag_in", [S // 2, D], BF16).ap()
            vag_out = nc.dram_tensor("vag_out", [S, D], BF16).ap()
            aps_head = (xT, xTq, wqT, wkT, wvT, masksT, ones, out,
                        ktag_in, ktag_out, vag_in, vag_out)
        with tile.TileContext(nc) as tc, ExitStack() as rctx:
            pspool = rctx.enter_context(
                tc.tile_pool(name="ps", bufs=2, space=bass.MemorySpace.PSUM)
            )
            _emit_body_v45(nc, tc, rctx, aps_head + (pspool,), version=version)
        nc.compile()
        return nc

    masks = nc.dram_tensor("masks", [P, NQT, 256], F32, kind="ExternalInput").ap()
    ident = nc.dram_tensor("ident", [P, P], BF16, kind="ExternalInput").ap()
    if version == 2:
        ktag_in = nc.dram_tensor("ktag_in", [D, S // 2], BF16).ap()
        ktag_out = nc.dram_tensor("ktag_out", [2 * D, S // 2], BF16).ap()
        vag_in = nc.dram_tensor("vag_in", [S // 2, D], BF16).ap()
        vag_out = nc.dram_tensor("vag_out", [S, D], BF16).ap()
        extra = (ktag_in, ktag_out, vag_in, vag_out)
        head = (xT,)
    elif version == 3:
        xTk = packed("xTk", S // 2)
        ktag_in = nc.dram_tensor("ktag_in", [D, S // 2], BF16).ap()
        ktag_out = nc.dram_tensor("ktag_out", [2 * D, S // 2], BF16).ap()
        extra = (ktag_in, ktag_out)
        head = (xT, xTk)
    else:
        extra = ()
        head = (xT,)

    with tile.TileContext(nc) as tc, ExitStack() as rctx:
        pspool = rctx.enter_context(
            tc.tile_pool(name="ps", bufs=2, space=bass.MemorySpace.PSUM)
        )
        aps = head + (xTq, wqT, wkT, wvT, masks, ident, out) + extra + (pspool,)
        _emit_body(nc, tc, rctx, aps, version=version)

    nc.compile()
    return nc


def _pack(wT, gw=512):
    # [1024, cols] -> [128, cols//gw, 8, gw], the SBUF tile layout:
    # pk[p, g, d, o] = wT[d*128 + p, g*gw + o]; contiguous per partition
    cols = wT.shape[1]
    w4 = wT.reshape(NDC, P, cols // gw, gw)
    return np.ascontiguousarray(w4.transpose(1, 2, 0, 3))


def _prep_inputs(x, Wk, Wq, Wv, version=1):
    x = np.asarray(x, dtype=np.float32)
    wqT = _pack((np.asarray(Wq, np.float32).T / 32.0).astype(NPBF16))
    wkT = _pack(np.asarray(Wk, np.float32).T.astype(NPBF16))
    wvT = _pack(np.asarray(Wv, np.float32).T.astype(NPBF16))
    ident = np.eye(P, dtype=NPBF16)

    mask_by_h = {}
    maskT_by_h = {}
    for h in (0, 1):
        mk = np.empty((P, NQT, 256), np.float32)
        for j, t in enumerate(TILES[h]):
            base = LJS[j] - 256
            col = base + np.arange(256)[None, :]
            row = t * P + np.arange(P)[:, None]
            mk[:, j, :] = np.where(col <= row, 0.0, -1e30)
        mask_by_h[h] = mk
        # transposed masks for v4/v5: maskT[kp, p, parity, qc] for
        # k-block kb = 2p + parity vs q-tile t = TILES[h][p]:
        # allowed iff kb*128 + kp <= t*128 + qc
        mt = np.empty((P, NQT, 2, P), np.float32)
        kp = np.arange(P)[:, None]
        qc = np.arange(P)[None, :]
        for p, t in enumerate(TILES[h]):
            for parity in (0, 1):
                kb = 2 * p + parity
                mt[:, p, parity, :] = np.where(
                    kb * P + kp <= t * P + qc, 0.0, -1e30
                )
        maskT_by_h[h] = mt

    in_maps = []
    for c in range(8):
        b, h = c // 2, c % 2
        xTb = np.ascontiguousarray(x[b].T.astype(NPBF16))
        qcols = np.concatenate([np.arange(t * P, (t + 1) * P) for t in TILES[h]])
        xt_in = (
            xTb
            if version in (1, 3, 4)
            else xTb[:, h * (S // 2) : (h + 1) * (S // 2)]
        )
        m = {
            "xT": _pack(xt_in),
            "xTq": _pack(xTb[:, qcols]),
            "wqT": wqT,
            "wkT": wkT,
            "wvT": wvT,
        }
        if version in (4, 5, 6):
            m["masksT"] = maskT_by_h[h]
            m["ones"] = np.ones((P, 1), np.float32)
        else:
            m["masks"] = mask_by_h[h]
            m["ident"] = ident
        if version in (3, 4):
            m["xTk"] = _pack(xTb[:, h * (S // 2) : (h + 1) * (S // 2)])
        if version == 6:
            m["hsel"] = np.array([[h]], np.uint32)
            m["prd"] = np.array([[1 - h]], np.uint32)
            m["koff0"] = np.array([[2 * h]], np.uint32)
            m["koff1"] = np.array([[2 * h + 1]], np.uint32)
        in_maps.append(m)
    return in_maps


VERSION = int(os.environ.get("BASS_KERNEL_VERSION", "4"))


def kernel(x, Wk, Wq, Wv):
    global LAST_RESULTS
    if VERSION not in _COMPILED:
        _COMPILED[VERSION] = _build(VERSION)
    nc = _COMPILED[VERSION]
    in_maps = _prep_inputs(x, Wk, Wq, Wv, version=VERSION)
    trace = bool(int(os.environ.get("BASS_KERNEL_TRACE", "0")))
    res = run_bass_kernel_spmd(nc, in_maps, list(range(8)), trace=trace)
    LAST_RESULTS = res
    out = np.empty((B, S, D), np.float32)
    for c in range(8):
        b, h = c // 2, c % 2
        oc = res.results[c]["out"]
        for j, t in enumerate(TILES[h]):
            out[b, t * P : (t + 1) * P, :] = oc[j * P : (j + 1) * P, :]
    return out

